# revision 21
# baseline (speedup 1.0000x reference)
"""ConvFlow (VITS-style coupling layer) Trainium2 kernel.

Data-parallel over 8 NeuronCores: 2 batch examples per core. Per core:
  x0 -> 1x1 pre-conv -> 4x [conv1d(k=5) -> LayerNorm(C) -> GELU] -> 1x1 proj
     -> rational-quadratic spline applied to x1; outputs (out, logdet).

Layout strategy:
  - Activations h live in SBUF as bf16 [cin(=128-part) x T(+4 pad)] tiles,
    4 channel-groups x 2 examples, ping-pong buffers between layers.
  - conv: psum[t(128), cout(512)] += sum_{cg,k} h[cg, t+k-2].T @ wT[k][cg]
    (+ K=1 ones-row matmul for the bias). LN stats on the psum tile
    (free-dim = channels), xhat=(x-mu)*rstd via one tensor_scalar -> bf16,
    PE-transpose 128x128 blocks back to [c, t], ACT gelu(g*x + b) with
    per-partition ln params writes the next h tile.
  - spline params in "mega" layout [128(t%128) x 32(t//128) x 2(half) x K]
    so every elementwise/bin op is a wide DVE/ACT op.
"""

import math
import numpy as np
import ml_dtypes

B, C_IN, T = 16, 4, 4096
HALF = C_IN // 2
F = 512
KS = 5
NL = 4
NB = 10            # NUM_BINS
TB = 5.0           # tail bound
MIN_BW = 1e-3
MIN_BH = 1e-3
MIN_D = 1e-3
EPS = 1e-5
NCORES = 8
BPC = B // NCORES  # examples per core = 2
NCG = F // 128     # channel groups = 4
NTT = T // 128     # token tiles per example = 32
NTB = T // 512     # 512-wide t banks = 8
TPAD = T + 4       # padded free dim for conv halo
SSCALE = 1.0 / math.sqrt(F)
NP3 = 3 * NB - 1   # 29 params per half
BF16 = ml_dtypes.bfloat16


def build_program(trivial_bias=True, trivial_mask=True):
    import concourse.bacc as bacc
    import concourse.tile as tile
    from concourse import mybir
    from contextlib import ExitStack

    dt = mybir.dt
    Alu = mybir.AluOpType
    Act = mybir.ActivationFunctionType
    Ax = mybir.AxisListType

    nc = bacc.Bacc("TRN2", target_bir_lowering=False, debug=False,
                   num_devices=NCORES)

    # ---- DRAM I/O ----
    def din(name, shape, dtype):
        return nc.dram_tensor(name, list(shape), dtype, kind="ExternalInput").ap()

    x_sh = din("x_sh", (BPC, C_IN, T), dt.float32)
    x0_bf = din("x0_bf", (BPC, HALF, T), dt.bfloat16)
    maskbc = din("maskbc", (BPC, 128, T), dt.bfloat16)
    maskmg = din("maskmg", (BPC, 128, NTT), dt.float32)
    prew_d = din("prew", (HALF, F), dt.bfloat16)
    preb_d = din("preb", (128, NCG), dt.float32)
    convw_d = din("convw", (NL, KS, F, F), dt.bfloat16)
    convb_d = din("convb", (NL, F), dt.bfloat16)
    lng_d = din("lng", (NL, 128, NCG), dt.float32)
    lnb_d = din("lnb", (NL, 128, NCG), dt.float32)
    projw_d = din("projw", (F, HALF * NP3), dt.bfloat16)
    projb_d = din("projb", (128, HALF * NP3), dt.float32)
    ident_d = din("ident", (128, 128), dt.bfloat16)
    iota_d = din("iota10", (128, NB), dt.float32)
    onesc_d = din("onesc", (1, 128), dt.bfloat16)
    onesf_d = din("onesf", (128, 1), dt.float32)

    out_sh = nc.dram_tensor("out_sh", [BPC, C_IN, T], dt.float32,
                            kind="ExternalOutput").ap()
    ld_sh = nc.dram_tensor("ld_sh", [BPC, 1], dt.float32,
                           kind="ExternalOutput").ap()

    with tile.TileContext(nc) as tc, ExitStack() as ctx:
        nv, ns, nt = nc.vector, nc.scalar, nc.tensor

        # ---------- constant / persistent pools ----------
        consts = ctx.enter_context(tc.tile_pool(name="consts", bufs=1))
        ident = consts.tile([128, 128], dt.bfloat16)
        nc.sync.dma_start(ident[:], ident_d[:])
        iota10 = consts.tile([128, NB], dt.float32)
        nc.sync.dma_start(iota10[:], iota_d[:])
        ones_col = consts.tile([1, 128], dt.bfloat16)
        nc.sync.dma_start(ones_col[:], onesc_d[:])
        ones_f = consts.tile([128, 1], dt.float32)
        nc.sync.dma_start(ones_f[:], onesf_d[:])
        eps_t = consts.tile([128, 1], dt.float32)
        nv.memset(eps_t[:], EPS)
        prew = consts.tile([HALF, F], dt.bfloat16)
        nc.sync.dma_start(prew[:], prew_d[:])
        preb = consts.tile([128, NCG], dt.float32)
        nc.sync.dma_start(preb[:], preb_d[:])
        convb = consts.tile([1, NL, F], dt.bfloat16)
        nc.sync.dma_start(convb[:], convb_d.unsqueeze(0))
        lng = consts.tile([128, NL, NCG], dt.float32)
        nc.sync.dma_start(lng[:], lng_d.rearrange("l p c -> p l c"))
        lnb = consts.tile([128, NL, NCG], dt.float32)
        nc.sync.dma_start(lnb[:], lnb_d.rearrange("l p c -> p l c"))
        projw = consts.tile([128, NCG, HALF * NP3], dt.bfloat16)
        nc.sync.dma_start(projw[:], projw_d.rearrange("(g p) n -> p g n", p=128))
        projb = consts.tile([128, HALF * NP3], dt.float32)
        nc.sync.dma_start(projb[:], projb_d[:])

        mask_bc = []
        mask_mg = []
        x_mega = []
        for e in range(BPC):
            if not trivial_mask:
                mb = consts.tile([128, T], dt.bfloat16, name=f"mask_bc{e}")
                nc.sync.dma_start(mb[:], maskbc[e])
                mask_bc.append(mb)
                mm = consts.tile([128, NTT], dt.float32, name=f"mask_mg{e}")
                nc.sync.dma_start(mm[:], maskmg[e])
                mask_mg.append(mm)
            xm = consts.tile([128, NTT, C_IN], dt.float32, name=f"x_mega{e}")
            for c in range(C_IN):
                nc.sync.dma_start(
                    xm[:, :, c:c + 1],
                    x_sh[e, c].rearrange("(j p) -> p j", p=128).unsqueeze(2))
            x_mega.append(xm)

        # spline params per example, [128, NTT, 58] f32
        params_pool = ctx.enter_context(tc.tile_pool(name="params", bufs=1))
        params = [params_pool.tile([128, NTT, HALF * NP3], dt.float32,
                                   name=f"params{e}") for e in range(BPC)]

        # ---------- psum + small work pools (live whole kernel) ----------
        psum_conv = ctx.enter_context(
            tc.tile_pool(name="psum_conv", bufs=2, space="PSUM"))
        psum_tp = ctx.enter_context(
            tc.tile_pool(name="psum_tp", bufs=3, space="PSUM"))
        psum_proj = ctx.enter_context(
            tc.tile_pool(name="psum_proj", bufs=2, space="PSUM"))
        stats = ctx.enter_context(tc.tile_pool(name="stats", bufs=2))
        xhat_pool = ctx.enter_context(tc.tile_pool(name="xhat", bufs=1))

        def conv_block(psum, h_src, e, j, wt, bias_row):
            """psum[t,cout] = sum_{cg,k} h.T @ w (+ ones.T @ bias)"""
            nmm = NCG * KS
            i = 0
            for cg in range(NCG):
                for k in range(KS):
                    lhsT = h_src[e][cg][:, j * 128 + k: j * 128 + k + 128]
                    nt.matmul(psum[:], lhsT, wt[cg][k][:], start=(i == 0),
                              stop=(trivial_bias and i == nmm - 1))
                    i += 1
            if not trivial_bias:
                nt.matmul(psum[:], ones_col[:], bias_row,
                          start=False, stop=True)

        def ln_chunk(h_dst, e, j):
            """Scratch slot for tile j's raw conv output (token-major),
            parked inside the destination buffer: chunk m=j//4 of tile
            cg=j%4. gelu writes of tile j only overwrite chunk j//4, whose
            four parked tiles (quad 4*(j//4)..+3) are consumed first."""
            m = j // 4
            return h_dst[e][j % 4][:, 2 + 512 * m: 2 + 512 * (m + 1)]

        def conv_ln_layer(e, l, src, dst, wt):
            """One conv+LN+gelu layer for example e."""
            mvbuf = stats.tile([128, NTT, 2], dt.float32, tag="mvbuf",
                               name=f"mvbuf{l}_{e}")
            # phase 1: conv -> stats; park raw conv out in dst chunks
            for j in range(NTT):
                ps = psum_conv.tile([128, F], dt.float32, tag="ps",
                                    name=f"ps{l}_{e}_{j}")
                conv_block(ps, src, e, j, wt, convb[:, l, :])
                st6 = stats.tile([128, 6], dt.float32, tag="st6",
                                 name=f"st6{l}_{e}_{j}")
                nv.bn_stats(st6[:], ps[:])
                nv.bn_aggr(mvbuf[:, j, :], st6[:])
                ns.copy(ln_chunk(dst, e, j), ps[:])
            # phase 2: one batched sqrt + reciprocal for all 32 tiles
            stdb = stats.tile([128, NTT], dt.float32, tag="stdb",
                              name=f"stdb{l}_{e}")
            ns.activation(stdb[:], mvbuf[:, :, 1:2], Act.Sqrt, bias=eps_t[:])
            rstdb = stats.tile([128, NTT], dt.float32, tag="rstdb",
                               name=f"rstdb{l}_{e}")
            nv.reciprocal(rstdb[:], stdb[:])
            # phase 3: per quad: xhat for 4 tiles, then transpose+gelu
            for q in range(NTT // 4):
                xh = []
                for jj in range(4):
                    j = 4 * q + jj
                    x = xhat_pool.tile([128, F], dt.bfloat16, tag=f"xh{jj}",
                                       name=f"xh{l}_{e}_{j}")
                    nv.tensor_scalar(x[:], ln_chunk(dst, e, j),
                                     mvbuf[:, j, 0:1], rstdb[:, j:j + 1],
                                     Alu.subtract, Alu.mult)
                    xh.append(x)
                for jj in range(4):
                    j = 4 * q + jj
                    for cg in range(NCG):
                        pt = psum_tp.tile([128, 128], dt.bfloat16, tag="pt",
                                          name=f"pt{l}_{e}_{j}_{cg}")
                        nt.matmul(pt[:], xh[jj][:, cg * 128:(cg + 1) * 128],
                                  ident[:], is_transpose=True)
                        ns.activation(
                            dst[e][cg][:, 2 + j * 128: 2 + (j + 1) * 128],
                            pt[:], Act.Gelu,
                            bias=lnb[:, l, cg:cg + 1],
                            scale=lng[:, l, cg:cg + 1])

        # ---------- activation ping-pong + weight pools ----------
        with tc.tile_pool(name="h_a", bufs=1) as h_a_pool:
            h_a = [[h_a_pool.tile([128, TPAD], dt.bfloat16, name=f"ha{e}_{cg}")
                    for cg in range(NCG)] for e in range(BPC)]
            with tc.tile_pool(name="h_b", bufs=1) as h_b_pool, \
                 tc.tile_pool(name="wpool", bufs=2) as wpool:
                h_b = [[h_b_pool.tile([128, TPAD], dt.bfloat16,
                                      name=f"hb{e}_{cg}")
                        for cg in range(NCG)] for e in range(BPC)]
                # zero the 2-col halos once; interiors are fully overwritten
                for hs in (h_a, h_b):
                    for e in range(BPC):
                        for cg in range(NCG):
                            nv.memset(hs[e][cg][:, 0:2], 0.0)
                            nv.memset(hs[e][cg][:, T + 2:T + 4], 0.0)

                def load_w(l):
                    wt = []
                    for cg in range(NCG):
                        row = []
                        for k in range(KS):
                            w = wpool.tile([128, F], dt.bfloat16,
                                           tag=f"w{cg}_{k}",
                                           bufs=2 if (cg < 3 and k < 4) else 1,
                                           name=f"w{l}_{cg}_{k}")
                            nc.sync.dma_start(
                                w[:],
                                convw_d[l, k, cg * 128:(cg + 1) * 128, :])
                            row.append(w)
                        wt.append(row)
                    return wt

                # ----- pre conv (1x1, K=2): x0 -> h_a, + bias, * mask -----
                wt_next = load_w(0)
                with tc.tile_pool(name="x0pool", bufs=2) as x0pool:
                    for e in range(BPC):
                        for jb in range(NTB):
                            x0c = x0pool.tile([HALF, 512], dt.bfloat16,
                                              tag="x0c", name=f"x0c{e}_{jb}")
                            nc.sync.dma_start(
                                x0c[:], x0_bf[e][:, jb * 512:(jb + 1) * 512])
                            for cg in range(NCG):
                                ps = psum_conv.tile([128, 512], dt.float32,
                                                    tag="ps")
                                nt.matmul(ps[:],
                                          prew[:, cg * 128:(cg + 1) * 128],
                                          x0c[:])
                                hdst = h_a[e][cg][:, 2 + jb * 512:
                                                  2 + (jb + 1) * 512]
                                if trivial_bias and trivial_mask:
                                    ns.copy(hdst, ps[:])
                                elif trivial_mask:
                                    nv.tensor_scalar(hdst, ps[:],
                                                     preb[:, cg:cg + 1], None,
                                                     Alu.add)
                                else:
                                    nv.scalar_tensor_tensor(
                                        hdst, ps[:], preb[:, cg:cg + 1],
                                        mask_bc[e][:, jb * 512:(jb + 1) * 512],
                                        Alu.add, Alu.mult)

                # ----- 4 conv layers -----
                src, dst = h_a, h_b
                for l in range(NL):
                    wt = wt_next
                    if l + 1 < NL:
                        wt_next = load_w(l + 1)
                    for e in range(BPC):
                        conv_ln_layer(e, l, src, dst, wt)
                        if (l < NL - 1) and not trivial_mask:
                            # mask for next conv's input
                            for cg in range(NCG):
                                nv.tensor_tensor(
                                    dst[e][cg][:, 2:2 + T],
                                    dst[e][cg][:, 2:2 + T],
                                    mask_bc[e][:],
                                    Alu.mult)
                    src, dst = dst, src
                # final h (gelu4 out, unmasked) now in `src`
                h4 = src

                # ----- proj conv (1x1) -> spline params -----
                for e in range(BPC):
                    for j in range(NTT):
                        pp = psum_proj.tile([128, HALF * NP3], dt.float32,
                                            tag="pp", name=f"pp{e}_{j}")
                        for cg in range(NCG):
                            nt.matmul(
                                pp[:],
                                h4[e][cg][:, 2 + j * 128: 2 + (j + 1) * 128],
                                projw[:, cg, :],
                                start=(cg == 0), stop=(cg == NCG - 1))
                        if trivial_bias and trivial_mask:
                            nv.tensor_copy(params[e][:, j, :], pp[:])
                        elif trivial_mask:
                            nv.tensor_tensor(params[e][:, j, :], pp[:],
                                             projb[:], Alu.add)
                        else:
                            tmp = stats.tile([128, HALF * NP3], dt.float32,
                                             tag="pj", name=f"pj{e}_{j}")
                            nv.tensor_tensor(tmp[:], pp[:], projb[:], Alu.add)
                            nv.tensor_scalar(params[e][:, j, :], tmp[:],
                                             mask_mg[e][:, j:j + 1], None,
                                             Alu.mult)
            # h_b + wpool released; h_a no longer needed either

        # h_a released here
        # ---------- spline phase ----------
        sp = ctx.enter_context(tc.tile_pool(name="spline", bufs=2))

        for e in range(BPC):
            P4 = params[e][:].rearrange("p j (h k) -> p j h k", h=HALF)
            x1m = x_mega[e][:, :, HALF:C_IN]            # [128, 32, 2]

            def t3(tag, k=1):
                if k == 1:
                    return sp.tile([128, NTT, HALF], dt.float32, tag=tag,
                                   name=f"sp_{tag}_{e}")
                return sp.tile([128, NTT, HALF, k], dt.float32, tag=tag,
                               name=f"sp_{tag}_{e}")

            def bcast_in(a):
                """[128,32,2] -> broadcast over innermost K dim."""
                return a.unsqueeze(3)

            # --- softmax(w), softmax(h) (no max-sub; inputs are small) ---
            ew = t3("ew", NB)
            ns.activation(ew[:], P4[:, :, :, 0:NB], Act.Exp, scale=SSCALE)
            eh = t3("eh", NB)
            ns.activation(eh[:], P4[:, :, :, NB:2 * NB], Act.Exp, scale=SSCALE)
            sw = t3("sw")
            nv.tensor_reduce(sw[:], ew[:], Ax.X, Alu.add)
            sh = t3("sh")
            nv.tensor_reduce(sh[:], eh[:], Ax.X, Alu.add)
            rw = t3("rw")
            nv.reciprocal(rw[:], sw[:])
            rh = t3("rh")
            nv.reciprocal(rh[:], sh[:])

            # cumsum buffers [.., 18]: cols 8..17 hold data
            ca = t3("ca", 18)
            cb = t3("cb", 18)
            cc = t3("cc", 18)
            cd = t3("cd", 18)

            def norm_cumsum(ebuf, rbuf, a, b, minb):
                # a[..,8:18] = minb + (1-minb*NB) * e * r
                nv.scalar_tensor_tensor(a[:, :, :, 8:18], ebuf[:],
                                        0.0, rbuf.unsqueeze(3).broadcast_to((128, NTT, HALF, NB)),
                                        Alu.add, Alu.mult)
                nv.tensor_scalar(a[:, :, :, 8:18], a[:, :, :, 8:18],
                                 1.0 - minb * NB, minb, Alu.mult, Alu.add)
                nv.memset(a[:, :, :, 0:8], 0.0)
                nv.memset(b[:, :, :, 0:8], 0.0)
                nv.tensor_tensor(b[:, :, :, 8:18], a[:, :, :, 8:18],
                                 a[:, :, :, 7:17], Alu.add)
                nv.tensor_tensor(a[:, :, :, 8:18], b[:, :, :, 8:18],
                                 b[:, :, :, 6:16], Alu.add)
                nv.tensor_tensor(b[:, :, :, 8:18], a[:, :, :, 8:18],
                                 a[:, :, :, 4:14], Alu.add)
                nv.tensor_tensor(a[:, :, :, 8:18], b[:, :, :, 8:18],
                                 b[:, :, :, 0:10], Alu.add)
                # a[..,8:17] = cumsum_1..9 ; build knots
                return a

            cwsum = norm_cumsum(ew, rw, ca, cb, MIN_BW)
            chsum = norm_cumsum(eh, rh, cc, cd, MIN_BH)

            cw = t3("cw", NB + 1)
            nv.tensor_scalar(cw[:, :, :, 1:NB], cwsum[:, :, :, 8:17],
                             2.0 * TB, -TB, Alu.mult, Alu.add)
            nv.memset(cw[:, :, :, 0:1], -TB)
            nv.memset(cw[:, :, :, NB:NB + 1], TB)
            ch = t3("ch", NB + 1)
            nv.tensor_scalar(ch[:, :, :, 1:NB], chsum[:, :, :, 8:17],
                             2.0 * TB, -TB, Alu.mult, Alu.add)
            nv.memset(ch[:, :, :, 0:1], -TB)
            nv.memset(ch[:, :, :, NB:NB + 1], TB)

            wbin = t3("wbin", NB)
            nv.tensor_tensor(wbin[:], cw[:, :, :, 1:NB + 1],
                             cw[:, :, :, 0:NB], Alu.subtract)
            hbin = t3("hbin", NB)
            nv.tensor_tensor(hbin[:], ch[:, :, :, 1:NB + 1],
                             ch[:, :, :, 0:NB], Alu.subtract)

            # --- derivatives d[0..10]: ends exactly 1.0 ---
            dd = t3("dd", NB + 1)
            ns.activation(dd[:, :, :, 1:NB], P4[:, :, :, 2 * NB:NP3],
                          Act.Exp)
            nv.tensor_scalar(dd[:, :, :, 1:NB], dd[:, :, :, 1:NB],
                             1.0, None, Alu.add)
            ns.activation(dd[:, :, :, 1:NB], dd[:, :, :, 1:NB], Act.Ln)
            nv.tensor_scalar(dd[:, :, :, 1:NB], dd[:, :, :, 1:NB],
                             MIN_D, None, Alu.add)
            nv.memset(dd[:, :, :, 0:1], 1.0)
            nv.memset(dd[:, :, :, NB:NB + 1], 1.0)

            # --- bin index ---
            xin = t3("xin")
            nv.tensor_scalar(xin[:], x1m, -TB, TB, Alu.max, Alu.min)
            ge = t3("ge", NB)
            nv.tensor_tensor(ge[:], bcast_in(xin[:]).broadcast_to(
                (128, NTT, HALF, NB)), cw[:, :, :, 0:NB], Alu.is_ge)
            idx = t3("idx")
            nv.tensor_reduce(idx[:], ge[:], Ax.X, Alu.add)
            nv.tensor_scalar(idx[:], idx[:], -1.0, None, Alu.add)
            oh = t3("oh", NB)
            nv.tensor_tensor(
                oh[:],
                iota10[:].unsqueeze(1).unsqueeze(1).broadcast_to(
                    (128, NTT, HALF, NB)),
                bcast_in(idx[:]).broadcast_to((128, NTT, HALF, NB)),
                Alu.is_equal)

            # --- gathers via one-hot ---
            def gather(src_ap, tag):
                t = t3("gt_" + tag, NB)
                nv.tensor_tensor(t[:], src_ap, oh[:], Alu.mult)
                g = t3("g_" + tag)
                nv.tensor_reduce(g[:], t[:], Ax.X, Alu.add)
                return g

            g_cw = gather(cw[:, :, :, 0:NB], "cw")
            g_w = gather(wbin[:], "w")
            g_ch = gather(ch[:, :, :, 0:NB], "ch")
            g_h = gather(hbin[:], "h")
            g_d = gather(dd[:, :, :, 0:NB], "d")
            g_d1 = gather(dd[:, :, :, 1:NB + 1], "d1")

            # --- rational quadratic ---
            rgw = t3("rgw")
            nv.reciprocal(rgw[:], g_w[:])
            delta = t3("delta")
            nv.tensor_tensor(delta[:], g_h[:], rgw[:], Alu.mult)
            theta = t3("theta")
            nv.tensor_tensor(theta[:], xin[:], g_cw[:], Alu.subtract)
            nv.tensor_tensor(theta[:], theta[:], rgw[:], Alu.mult)
            omt = t3("omt")
            nv.tensor_scalar(omt[:], theta[:], -1.0, 1.0, Alu.mult, Alu.add)
            tomt = t3("tomt")
            nv.tensor_tensor(tomt[:], theta[:], omt[:], Alu.mult)
            th2 = t3("th2")
            nv.tensor_tensor(th2[:], theta[:], theta[:], Alu.mult)
            omt2 = t3("omt2")
            nv.tensor_tensor(omt2[:], omt[:], omt[:], Alu.mult)

            # num = g_h * (delta*th2 + g_d*tomt)
            t_a = t3("t_a")
            nv.tensor_tensor(t_a[:], delta[:], th2[:], Alu.mult)
            t_b = t3("t_b")
            nv.tensor_tensor(t_b[:], g_d[:], tomt[:], Alu.mult)
            nv.tensor_tensor(t_a[:], t_a[:], t_b[:], Alu.add)
            num = t3("num")
            nv.tensor_tensor(num[:], g_h[:], t_a[:], Alu.mult)
            # den = delta + (g_d + g_d1 - 2*delta) * tomt
            t_c = t3("t_c")
            nv.tensor_tensor(t_c[:], g_d[:], g_d1[:], Alu.add)
            nv.scalar_tensor_tensor(t_c[:], delta[:], -2.0, t_c[:],
                                    Alu.mult, Alu.add)
            nv.tensor_tensor(t_c[:], t_c[:], tomt[:], Alu.mult)
            den = t3("den")
            nv.tensor_tensor(den[:], t_c[:], delta[:], Alu.add)
            rden = t3("rden")
            nv.reciprocal(rden[:], den[:])
            outv = t3("outv")
            nv.tensor_tensor(outv[:], num[:], rden[:], Alu.mult)
            nv.tensor_tensor(outv[:], outv[:], g_ch[:], Alu.add)

            # dnum = delta^2*(g_d1*th2 + 2*delta*tomt + g_d*omt2)
            t_d = t3("t_d")
            nv.tensor_tensor(t_d[:], g_d1[:], th2[:], Alu.mult)
            t_e = t3("t_e")
            nv.scalar_tensor_tensor(t_e[:], delta[:], 2.0, tomt[:],
                                    Alu.mult, Alu.mult)
            nv.tensor_tensor(t_d[:], t_d[:], t_e[:], Alu.add)
            t_f = t3("t_f")
            nv.tensor_tensor(t_f[:], g_d[:], omt2[:], Alu.mult)
            nv.tensor_tensor(t_d[:], t_d[:], t_f[:], Alu.add)
            d2 = t3("d2")
            nv.tensor_tensor(d2[:], delta[:], delta[:], Alu.mult)
            nv.tensor_tensor(t_d[:], t_d[:], d2[:], Alu.mult)
            # lad = ln(dnum * rden^2)
            nv.tensor_tensor(t_d[:], t_d[:], rden[:], Alu.mult)
            nv.tensor_tensor(t_d[:], t_d[:], rden[:], Alu.mult)
            lad = t3("lad")
            ns.activation(lad[:], t_d[:], Act.Ln)

            # --- inside mask + select ---
            ins1 = t3("ins1")
            nv.tensor_scalar(ins1[:], x1m, -TB, None, Alu.is_ge)
            ins2 = t3("ins2")
            nv.tensor_scalar(ins2[:], x1m, TB, None, Alu.is_le)
            inside = t3("inside")
            nv.tensor_tensor(inside[:], ins1[:], ins2[:], Alu.mult)

            inside_i = sp.tile([128, NTT, HALF], dt.int32, tag="inside_i",
                               name=f"sp_inside_i_{e}")
            nv.tensor_copy(inside_i[:], inside[:])
            x1n = t3("x1n")
            nv.tensor_copy(x1n[:], x1m)
            nv.copy_predicated(x1n[:], inside_i[:], outv[:])

            # masked outputs
            if trivial_mask:
                x0m = x_mega[e][:, :, 0:HALF]
            else:
                mgb = mask_mg[e][:].unsqueeze(2).broadcast_to(
                    (128, NTT, HALF))
                x0m_t = t3("x0m")
                nv.tensor_tensor(x0m_t[:], x_mega[e][:, :, 0:HALF], mgb,
                                 Alu.mult)
                nv.tensor_tensor(x1n[:], x1n[:], mgb, Alu.mult)
                x0m = x0m_t[:]

            out_r = out_sh[e].rearrange("c (j p) -> p j c", p=128)
            for hc in range(HALF):
                nc.sync.dma_start(out_r[:, :, hc:hc + 1],
                                  x0m[:, :, hc:hc + 1])
                nc.sync.dma_start(out_r[:, :, HALF + hc:HALF + hc + 1],
                                  x1n[:, :, hc:hc + 1])

            # --- logdet ---
            ladm = t3("ladm")
            nv.tensor_tensor(ladm[:], lad[:], inside[:], Alu.mult)
            if not trivial_mask:
                nv.tensor_tensor(ladm[:], ladm[:], mgb, Alu.mult)
            ldp = sp.tile([128, 1], dt.float32, tag="ldp", name=f"ldp{e}")
            nv.tensor_reduce(ldp[:], ladm[:], Ax.XY, Alu.add)
            pl = psum_proj.tile([1, 1], dt.float32, tag="pl", bufs=1,
                                name=f"pl{e}")
            nt.matmul(pl[:], ldp[:], ones_f[:])
            lds = sp.tile([1, 1], dt.float32, tag="lds", name=f"lds{e}")
            ns.copy(lds[:], pl[:])
            nc.sync.dma_start(ld_sh[e], lds[:])

    nc.compile()
    return nc


_prog_cache = {}


def _triviality(inputs):
    """Host-visible input properties the program specializes on."""
    tb = (not np.any(np.asarray(inputs["pre_b"]))
          and not np.any(np.asarray(inputs["conv_b"]))
          and not np.any(np.asarray(inputs["proj_b"])))
    tm = bool(np.all(np.asarray(inputs["x_mask"]) == 1.0))
    return tb, tm


def _get_program(trivial_bias=True, trivial_mask=True):
    key = (trivial_bias, trivial_mask)
    if key not in _prog_cache:
        _prog_cache[key] = build_program(trivial_bias, trivial_mask)
    return _prog_cache[key]


def make_in_maps(x, x_mask, pre_w, pre_b, conv_w, conv_b, ln_g, ln_b,
                 proj_w, proj_b):
    x = np.asarray(x, np.float32)
    x_mask = np.asarray(x_mask, np.float32)
    prew = np.asarray(pre_w, np.float32).reshape(F, HALF).T.astype(BF16)
    preb = np.asarray(pre_b, np.float32).reshape(NCG, 128).T.copy()
    convw = np.transpose(np.asarray(conv_w, np.float32),
                         (0, 3, 2, 1)).astype(BF16).copy()
    convb = np.asarray(conv_b, np.float32).astype(BF16)
    lng = np.asarray(ln_g, np.float32).reshape(NL, NCG, 128).transpose(
        0, 2, 1).copy()
    lnb = np.asarray(ln_b, np.float32).reshape(NL, NCG, 128).transpose(
        0, 2, 1).copy()
    projw = np.asarray(proj_w, np.float32).reshape(HALF * NP3, F).T.astype(
        BF16).copy()
    projb = np.tile(np.asarray(proj_b, np.float32)[None, :], (128, 1)).copy()
    ident = np.eye(128, dtype=BF16)
    iota = np.tile(np.arange(NB, dtype=np.float32)[None, :], (128, 1)).copy()
    onesc = np.ones((1, 128), BF16)
    onesf = np.ones((128, 1), np.float32)

    in_maps = []
    for c in range(NCORES):
        sl = slice(c * BPC, (c + 1) * BPC)
        xs = x[sl]
        ms = x_mask[sl]
        in_maps.append({
            "x_sh": xs.copy(),
            "x0_bf": xs[:, :HALF, :].astype(BF16).copy(),
            "maskbc": np.tile(ms[:, 0:1, :].astype(BF16), (1, 128, 1)).copy(),
            "maskmg": np.ascontiguousarray(
                ms[:, 0, :].reshape(BPC, NTT, 128).transpose(0, 2, 1)),
            "prew": prew.copy(), "preb": preb, "convw": convw,
            "convb": convb, "lng": lng, "lnb": lnb,
            "projw": projw, "projb": projb, "ident": ident,
            "iota10": iota, "onesc": onesc, "onesf": onesf,
        })
    return in_maps


def kernel(**inputs):
    from concourse.bass_utils import run_bass_kernel_spmd
    tb, tm = _triviality(inputs)
    nc = _get_program(tb, tm)
    in_maps = make_in_maps(**inputs)
    res = run_bass_kernel_spmd(nc, in_maps, list(range(NCORES))).results
    out = np.concatenate([r["out_sh"] for r in res], axis=0)
    logdet = np.concatenate([r["ld_sh"].reshape(BPC) for r in res], axis=0)
    return out.astype(np.float32), logdet.astype(np.float32)


# revision 26
# speedup vs baseline: 1.1230x; 1.1230x over previous
"""ConvFlow (VITS-style coupling layer) Trainium2 kernel.

Data-parallel over 8 NeuronCores: 2 batch examples per core. Per core:
  x0 -> 1x1 pre-conv -> 4x [conv1d(k=5) -> LayerNorm(C) -> GELU] -> 1x1 proj
     -> rational-quadratic spline applied to x1; outputs (out, logdet).

Layout strategy:
  - Activations h live in SBUF as bf16 [cin(=128-part) x T(+4 pad)] tiles,
    4 channel-groups x 2 examples, ping-pong buffers between layers.
  - conv: psum[t(128), cout(512)] += sum_{cg,k} h[cg, t+k-2].T @ wT[k][cg]
    (+ K=1 ones-row matmul for the bias). LN stats on the psum tile
    (free-dim = channels), xhat=(x-mu)*rstd via one tensor_scalar -> bf16,
    PE-transpose 128x128 blocks back to [c, t], ACT gelu(g*x + b) with
    per-partition ln params writes the next h tile.
  - spline params in "mega" layout [128(t%128) x 32(t//128) x 2(half) x K]
    so every elementwise/bin op is a wide DVE/ACT op.
"""

import math
import numpy as np
import ml_dtypes

B, C_IN, T = 16, 4, 4096
HALF = C_IN // 2
F = 512
KS = 5
NL = 4
NB = 10            # NUM_BINS
TB = 5.0           # tail bound
MIN_BW = 1e-3
MIN_BH = 1e-3
MIN_D = 1e-3
EPS = 1e-5
NCORES = 8
BPC = B // NCORES  # examples per core = 2
NCG = F // 128     # channel groups = 4
NTT = T // 128     # token tiles per example = 32
NTB = T // 512     # 512-wide t banks = 8
TPAD = T + 4       # padded free dim for conv halo
SSCALE = 1.0 / math.sqrt(F)
NP3 = 3 * NB - 1   # 29 params per half
BF16 = ml_dtypes.bfloat16


def build_program(trivial_bias=True, trivial_mask=True):
    import concourse.bacc as bacc
    import concourse.tile as tile
    from concourse import mybir
    from contextlib import ExitStack

    dt = mybir.dt
    Alu = mybir.AluOpType
    Act = mybir.ActivationFunctionType
    Ax = mybir.AxisListType

    nc = bacc.Bacc("TRN2", target_bir_lowering=False, debug=False,
                   num_devices=NCORES)

    # ---- DRAM I/O ----
    def din(name, shape, dtype):
        return nc.dram_tensor(name, list(shape), dtype, kind="ExternalInput").ap()

    x_sh = din("x_sh", (BPC, C_IN, T), dt.float32)
    x0_bf = din("x0_bf", (BPC, HALF, T), dt.bfloat16)
    maskbc = din("maskbc", (BPC, 128, T), dt.bfloat16)
    maskmg = din("maskmg", (BPC, 128, NTT), dt.float32)
    prew_d = din("prew", (HALF, F), dt.bfloat16)
    preb_d = din("preb", (128, NCG), dt.float32)
    convw_d = din("convw", (NL, KS, F, F), dt.bfloat16)
    convb_d = din("convb", (NL, F), dt.bfloat16)
    lng_d = din("lng", (NL, 128, NCG), dt.float32)
    lnb_d = din("lnb", (NL, 128, NCG), dt.float32)
    projw_d = din("projw", (F, HALF * NP3), dt.bfloat16)
    projb_d = din("projb", (128, HALF * NP3), dt.float32)
    ident_d = din("ident", (128, 128), dt.bfloat16)
    iota_d = din("iota10", (128, NB), dt.float32)
    onesc_d = din("onesc", (1, 128), dt.bfloat16)
    onesf_d = din("onesf", (128, 1), dt.float32)

    out_sh = nc.dram_tensor("out_sh", [BPC, C_IN, T], dt.float32,
                            kind="ExternalOutput").ap()
    ld_sh = nc.dram_tensor("ld_sh", [BPC, 1], dt.float32,
                           kind="ExternalOutput").ap()

    with tile.TileContext(nc) as tc, ExitStack() as ctx:
        nv, ns, nt = nc.vector, nc.scalar, nc.tensor

        # ---------- constant / persistent pools ----------
        consts = ctx.enter_context(tc.tile_pool(name="consts", bufs=1))
        ident = consts.tile([128, 128], dt.bfloat16)
        nc.sync.dma_start(ident[:], ident_d[:])
        iota10 = consts.tile([128, NB], dt.float32)
        nc.sync.dma_start(iota10[:], iota_d[:])
        ones_col = consts.tile([1, 128], dt.bfloat16)
        nc.sync.dma_start(ones_col[:], onesc_d[:])
        ones_f = consts.tile([128, 1], dt.float32)
        nc.sync.dma_start(ones_f[:], onesf_d[:])
        eps_t = consts.tile([128, 1], dt.float32)
        nv.memset(eps_t[:], EPS)
        prew = consts.tile([HALF, F], dt.bfloat16)
        nc.sync.dma_start(prew[:], prew_d[:])
        preb = consts.tile([128, NCG], dt.float32)
        nc.sync.dma_start(preb[:], preb_d[:])
        convb = consts.tile([1, NL, F], dt.bfloat16)
        nc.sync.dma_start(convb[:], convb_d.unsqueeze(0))
        lng = consts.tile([128, NL, NCG], dt.float32)
        nc.sync.dma_start(lng[:], lng_d.rearrange("l p c -> p l c"))
        lnb = consts.tile([128, NL, NCG], dt.float32)
        nc.sync.dma_start(lnb[:], lnb_d.rearrange("l p c -> p l c"))
        projw = consts.tile([128, NCG, HALF * NP3], dt.bfloat16)
        nc.sync.dma_start(projw[:], projw_d.rearrange("(g p) n -> p g n", p=128))
        projb = consts.tile([128, HALF * NP3], dt.float32)
        nc.sync.dma_start(projb[:], projb_d[:])

        mask_bc = []
        mask_mg = []
        x_mega = []
        for e in range(BPC):
            if not trivial_mask:
                mb = consts.tile([128, T], dt.bfloat16, name=f"mask_bc{e}")
                nc.sync.dma_start(mb[:], maskbc[e])
                mask_bc.append(mb)
                mm = consts.tile([128, NTT], dt.float32, name=f"mask_mg{e}")
                nc.sync.dma_start(mm[:], maskmg[e])
                mask_mg.append(mm)
            xm = consts.tile([128, NTT, C_IN], dt.float32, name=f"x_mega{e}")
            for c in range(C_IN):
                nc.sync.dma_start(
                    xm[:, :, c:c + 1],
                    x_sh[e, c].rearrange("(j p) -> p j", p=128).unsqueeze(2))
            x_mega.append(xm)

        # spline params per example, [128, NTT, 58] f32
        params_pool = ctx.enter_context(tc.tile_pool(name="params", bufs=1))
        params = [params_pool.tile([128, NTT, HALF * NP3], dt.float32,
                                   name=f"params{e}") for e in range(BPC)]

        # ---------- psum + small work pools (live whole kernel) ----------
        psum_conv = ctx.enter_context(
            tc.tile_pool(name="psum_conv", bufs=2, space="PSUM"))
        psum_tp = ctx.enter_context(
            tc.tile_pool(name="psum_tp", bufs=3, space="PSUM"))
        psum_proj = ctx.enter_context(
            tc.tile_pool(name="psum_proj", bufs=2, space="PSUM"))
        stats = ctx.enter_context(tc.tile_pool(name="stats", bufs=2))
        xhat_pool = ctx.enter_context(tc.tile_pool(name="xhat", bufs=1))

        def conv_block(psum, h_src, e, j, wt, bias_row):
            """psum[t,cout] = sum_{cg,k} h.T @ w (+ ones.T @ bias)"""
            nmm = NCG * KS
            i = 0
            for cg in range(NCG):
                for k in range(KS):
                    lhsT = h_src[e][cg][:, j * 128 + k: j * 128 + k + 128]
                    nt.matmul(psum[:], lhsT, wt[cg][k][:], start=(i == 0),
                              stop=(trivial_bias and i == nmm - 1))
                    i += 1
            if not trivial_bias:
                nt.matmul(psum[:], ones_col[:], bias_row,
                          start=False, stop=True)

        def ln_chunk(h_dst, e, j):
            """Scratch slot for tile j's raw conv output (token-major),
            parked inside the destination buffer: chunk m=j//4 of tile
            cg=j%4. gelu writes of tile j only overwrite chunk j//4, whose
            four parked tiles (quad 4*(j//4)..+3) are consumed first."""
            m = j // 4
            return h_dst[e][j % 4][:, 2 + 512 * m: 2 + 512 * (m + 1)]

        def conv_phase1(e, l, src, dst, wt):
            """conv -> park raw out in dst chunks; sum/sumsq via ACT accum."""
            sb = stats.tile([128, NTT], dt.float32, tag="sumb",
                            name=f"sumb{l}_{e}")
            sqb = stats.tile([128, NTT], dt.float32, tag="sqb",
                             name=f"sqb{l}_{e}")
            for j in range(NTT):
                ps = psum_conv.tile([128, F], dt.float32, tag="ps",
                                    name=f"ps{l}_{e}_{j}")
                conv_block(ps, src, e, j, wt, convb[:, l, :])
                ns.activation(ln_chunk(dst, e, j), ps[:], Act.Copy,
                              accum_out=sb[:, j:j + 1])
                sqd = stats.tile([128, F], dt.bfloat16, tag="sqd",
                                 name=f"sqd{l}_{e}_{j}")
                ns.activation(sqd[:], ps[:], Act.Square,
                              accum_out=sqb[:, j:j + 1])
            return sb, sqb

        def conv_phase23(e, l, dst, sb, sqb):
            """Batched mean/rstd, then per-quad xhat -> transpose -> gelu."""
            meanb = stats.tile([128, NTT], dt.float32, tag="meanb",
                               name=f"meanb{l}_{e}")
            nv.tensor_scalar(meanb[:], sb[:], 1.0 / F, None, Alu.mult)
            varb = stats.tile([128, NTT], dt.float32, tag="varb",
                              name=f"varb{l}_{e}")
            # var = sumsq/F - mean^2
            nv.scalar_tensor_tensor(varb[:], meanb[:], 0.0, meanb[:],
                                    Alu.add, Alu.mult)
            nv.scalar_tensor_tensor(varb[:], sqb[:], 1.0 / F, varb[:],
                                    Alu.mult, Alu.subtract)
            stdb = stats.tile([128, NTT], dt.float32, tag="stdb",
                              name=f"stdb{l}_{e}")
            ns.activation(stdb[:], varb[:], Act.Sqrt, bias=eps_t[:])
            rstdb = stats.tile([128, NTT], dt.float32, tag="rstdb",
                               name=f"rstdb{l}_{e}")
            nv.reciprocal(rstdb[:], stdb[:])
            for q in range(NTT // 4):
                xh = []
                for jj in range(4):
                    j = 4 * q + jj
                    x = xhat_pool.tile([128, F], dt.bfloat16, tag=f"xh{jj}",
                                       name=f"xh{l}_{e}_{j}")
                    nv.tensor_scalar(x[:], ln_chunk(dst, e, j),
                                     meanb[:, j:j + 1], rstdb[:, j:j + 1],
                                     Alu.subtract, Alu.mult)
                    xh.append(x)
                for cg in range(NCG):
                    ptb = psum_tp.tile([128, 512], dt.bfloat16, tag="pt",
                                       name=f"pt{l}_{e}_{q}_{cg}")
                    for jj in range(4):
                        nt.matmul(ptb[:, jj * 128:(jj + 1) * 128],
                                  xh[jj][:, cg * 128:(cg + 1) * 128],
                                  ident[:], is_transpose=True)
                    ns.activation(
                        dst[e][cg][:, 2 + 512 * q: 2 + 512 * (q + 1)],
                        ptb[:], Act.Gelu,
                        bias=lnb[:, l, cg:cg + 1],
                        scale=lng[:, l, cg:cg + 1])

        # ---------- activation ping-pong + weight pools ----------
        with tc.tile_pool(name="h_a", bufs=1) as h_a_pool:
            h_a = [[h_a_pool.tile([128, TPAD], dt.bfloat16, name=f"ha{e}_{cg}")
                    for cg in range(NCG)] for e in range(BPC)]
            with tc.tile_pool(name="h_b", bufs=1) as h_b_pool, \
                 tc.tile_pool(name="wpool", bufs=2) as wpool:
                h_b = [[h_b_pool.tile([128, TPAD], dt.bfloat16,
                                      name=f"hb{e}_{cg}")
                        for cg in range(NCG)] for e in range(BPC)]
                # zero the 2-col halos once; interiors are fully overwritten
                for hs in (h_a, h_b):
                    for e in range(BPC):
                        for cg in range(NCG):
                            nv.memset(hs[e][cg][:, 0:2], 0.0)
                            nv.memset(hs[e][cg][:, T + 2:T + 4], 0.0)

                def load_w(l):
                    wt = []
                    for cg in range(NCG):
                        row = []
                        for k in range(KS):
                            wb = 2 if trivial_mask else (
                                2 if (cg < 3 and k < 4) else 1)
                            w = wpool.tile([128, F], dt.bfloat16,
                                           tag=f"w{cg}_{k}", bufs=wb,
                                           name=f"w{l}_{cg}_{k}")
                            nc.sync.dma_start(
                                w[:],
                                convw_d[l, k, cg * 128:(cg + 1) * 128, :])
                            row.append(w)
                        wt.append(row)
                    return wt

                # ----- pre conv (1x1, K=2): x0 -> h_a, + bias, * mask -----
                wt_next = load_w(0)
                with tc.tile_pool(name="x0pool",
                                  bufs=2 if trivial_mask else 1) as x0pool:
                    for e in range(BPC):
                        for jb in range(NTB):
                            x0c = x0pool.tile([HALF, 512], dt.bfloat16,
                                              tag="x0c", name=f"x0c{e}_{jb}")
                            nc.sync.dma_start(
                                x0c[:], x0_bf[e][:, jb * 512:(jb + 1) * 512])
                            for cg in range(NCG):
                                ps = psum_conv.tile([128, 512], dt.float32,
                                                    tag="ps")
                                nt.matmul(ps[:],
                                          prew[:, cg * 128:(cg + 1) * 128],
                                          x0c[:])
                                hdst = h_a[e][cg][:, 2 + jb * 512:
                                                  2 + (jb + 1) * 512]
                                if trivial_bias and trivial_mask:
                                    ns.copy(hdst, ps[:])
                                elif trivial_mask:
                                    nv.tensor_scalar(hdst, ps[:],
                                                     preb[:, cg:cg + 1], None,
                                                     Alu.add)
                                else:
                                    nv.scalar_tensor_tensor(
                                        hdst, ps[:], preb[:, cg:cg + 1],
                                        mask_bc[e][:, jb * 512:(jb + 1) * 512],
                                        Alu.add, Alu.mult)

                # ----- 4 conv layers -----
                src, dst = h_a, h_b
                for l in range(NL):
                    wt = wt_next
                    if l + 1 < NL:
                        wt_next = load_w(l + 1)
                    st = [None, None]
                    for e in range(BPC):
                        st[e] = conv_phase1(e, l, src, dst, wt)
                    for e in range(BPC):
                        conv_phase23(e, l, dst, *st[e])
                    for e in range(BPC):
                        if (l < NL - 1) and not trivial_mask:
                            # mask for next conv's input
                            for cg in range(NCG):
                                nv.tensor_tensor(
                                    dst[e][cg][:, 2:2 + T],
                                    dst[e][cg][:, 2:2 + T],
                                    mask_bc[e][:],
                                    Alu.mult)
                    src, dst = dst, src
                # final h (gelu4 out, unmasked) now in `src`
                h4 = src

                # ----- proj conv (1x1) -> spline params -----
                for e in range(BPC):
                    for j in range(NTT):
                        pp = psum_proj.tile([128, HALF * NP3], dt.float32,
                                            tag="pp", name=f"pp{e}_{j}")
                        for cg in range(NCG):
                            nt.matmul(
                                pp[:],
                                h4[e][cg][:, 2 + j * 128: 2 + (j + 1) * 128],
                                projw[:, cg, :],
                                start=(cg == 0), stop=(cg == NCG - 1))
                        if trivial_bias and trivial_mask:
                            nv.tensor_copy(params[e][:, j, :], pp[:])
                        elif trivial_mask:
                            nv.tensor_tensor(params[e][:, j, :], pp[:],
                                             projb[:], Alu.add)
                        else:
                            tmp = stats.tile([128, HALF * NP3], dt.float32,
                                             tag="pj", name=f"pj{e}_{j}")
                            nv.tensor_tensor(tmp[:], pp[:], projb[:], Alu.add)
                            nv.tensor_scalar(params[e][:, j, :], tmp[:],
                                             mask_mg[e][:, j:j + 1], None,
                                             Alu.mult)
            # h_b + wpool released; h_a no longer needed either

        # h_a released here
        # ---------- spline phase ----------
        sp = ctx.enter_context(tc.tile_pool(name="spline", bufs=2))

        for e in range(BPC):
            P4 = params[e][:].rearrange("p j (h k) -> p j h k", h=HALF)
            x1m = x_mega[e][:, :, HALF:C_IN]            # [128, 32, 2]

            def t3(tag, k=1):
                if k == 1:
                    return sp.tile([128, NTT, HALF], dt.float32, tag=tag,
                                   name=f"sp_{tag}_{e}")
                return sp.tile([128, NTT, HALF, k], dt.float32, tag=tag,
                               name=f"sp_{tag}_{e}")

            def bcast_in(a):
                """[128,32,2] -> broadcast over innermost K dim."""
                return a.unsqueeze(3)

            # --- softmax(w), softmax(h) (no max-sub; inputs are small) ---
            ew = t3("ew", NB)
            ns.activation(ew[:], P4[:, :, :, 0:NB], Act.Exp, scale=SSCALE)
            eh = t3("eh", NB)
            ns.activation(eh[:], P4[:, :, :, NB:2 * NB], Act.Exp, scale=SSCALE)
            sw = t3("sw")
            nv.tensor_reduce(sw[:], ew[:], Ax.X, Alu.add)
            sh = t3("sh")
            nv.tensor_reduce(sh[:], eh[:], Ax.X, Alu.add)
            rw = t3("rw")
            nv.reciprocal(rw[:], sw[:])
            rh = t3("rh")
            nv.reciprocal(rh[:], sh[:])

            # cumsum buffers [.., 18]: cols 8..17 hold data
            ca = t3("ca", 18)
            cb = t3("cb", 18)
            cc = t3("cc", 18)
            cd = t3("cd", 18)

            def norm_cumsum(ebuf, rbuf, a, b, minb):
                # a[..,8:18] = minb + (1-minb*NB) * e * r
                nv.scalar_tensor_tensor(a[:, :, :, 8:18], ebuf[:],
                                        0.0, rbuf.unsqueeze(3).broadcast_to((128, NTT, HALF, NB)),
                                        Alu.add, Alu.mult)
                nv.tensor_scalar(a[:, :, :, 8:18], a[:, :, :, 8:18],
                                 1.0 - minb * NB, minb, Alu.mult, Alu.add)
                nv.memset(a[:, :, :, 0:8], 0.0)
                nv.memset(b[:, :, :, 0:8], 0.0)
                nv.tensor_tensor(b[:, :, :, 8:18], a[:, :, :, 8:18],
                                 a[:, :, :, 7:17], Alu.add)
                nv.tensor_tensor(a[:, :, :, 8:18], b[:, :, :, 8:18],
                                 b[:, :, :, 6:16], Alu.add)
                nv.tensor_tensor(b[:, :, :, 8:18], a[:, :, :, 8:18],
                                 a[:, :, :, 4:14], Alu.add)
                nv.tensor_tensor(a[:, :, :, 8:18], b[:, :, :, 8:18],
                                 b[:, :, :, 0:10], Alu.add)
                # a[..,8:17] = cumsum_1..9 ; build knots
                return a

            cwsum = norm_cumsum(ew, rw, ca, cb, MIN_BW)
            chsum = norm_cumsum(eh, rh, cc, cd, MIN_BH)

            cw = t3("cw", NB + 1)
            nv.tensor_scalar(cw[:, :, :, 1:NB], cwsum[:, :, :, 8:17],
                             2.0 * TB, -TB, Alu.mult, Alu.add)
            nv.memset(cw[:, :, :, 0:1], -TB)
            nv.memset(cw[:, :, :, NB:NB + 1], TB)
            ch = t3("ch", NB + 1)
            nv.tensor_scalar(ch[:, :, :, 1:NB], chsum[:, :, :, 8:17],
                             2.0 * TB, -TB, Alu.mult, Alu.add)
            nv.memset(ch[:, :, :, 0:1], -TB)
            nv.memset(ch[:, :, :, NB:NB + 1], TB)

            wbin = t3("wbin", NB)
            nv.tensor_tensor(wbin[:], cw[:, :, :, 1:NB + 1],
                             cw[:, :, :, 0:NB], Alu.subtract)
            hbin = t3("hbin", NB)
            nv.tensor_tensor(hbin[:], ch[:, :, :, 1:NB + 1],
                             ch[:, :, :, 0:NB], Alu.subtract)

            # --- derivatives d[0..10]: ends exactly 1.0 ---
            dd = t3("dd", NB + 1)
            ns.activation(dd[:, :, :, 1:NB], P4[:, :, :, 2 * NB:NP3],
                          Act.Exp)
            nv.tensor_scalar(dd[:, :, :, 1:NB], dd[:, :, :, 1:NB],
                             1.0, None, Alu.add)
            ns.activation(dd[:, :, :, 1:NB], dd[:, :, :, 1:NB], Act.Ln)
            nv.tensor_scalar(dd[:, :, :, 1:NB], dd[:, :, :, 1:NB],
                             MIN_D, None, Alu.add)
            nv.memset(dd[:, :, :, 0:1], 1.0)
            nv.memset(dd[:, :, :, NB:NB + 1], 1.0)

            # --- bin index ---
            xin = t3("xin")
            nv.tensor_scalar(xin[:], x1m, -TB, TB, Alu.max, Alu.min)
            ge = t3("ge", NB)
            nv.tensor_tensor(ge[:], bcast_in(xin[:]).broadcast_to(
                (128, NTT, HALF, NB)), cw[:, :, :, 0:NB], Alu.is_ge)
            idx = t3("idx")
            nv.tensor_reduce(idx[:], ge[:], Ax.X, Alu.add)
            nv.tensor_scalar(idx[:], idx[:], -1.0, None, Alu.add)
            oh = t3("oh", NB)
            nv.tensor_tensor(
                oh[:],
                iota10[:].unsqueeze(1).unsqueeze(1).broadcast_to(
                    (128, NTT, HALF, NB)),
                bcast_in(idx[:]).broadcast_to((128, NTT, HALF, NB)),
                Alu.is_equal)

            # --- gathers via one-hot ---
            def gather(src_ap, tag):
                t = t3("gt_" + tag, NB)
                nv.tensor_tensor(t[:], src_ap, oh[:], Alu.mult)
                g = t3("g_" + tag)
                nv.tensor_reduce(g[:], t[:], Ax.X, Alu.add)
                return g

            g_cw = gather(cw[:, :, :, 0:NB], "cw")
            g_w = gather(wbin[:], "w")
            g_ch = gather(ch[:, :, :, 0:NB], "ch")
            g_h = gather(hbin[:], "h")
            g_d = gather(dd[:, :, :, 0:NB], "d")
            g_d1 = gather(dd[:, :, :, 1:NB + 1], "d1")

            # --- rational quadratic ---
            rgw = t3("rgw")
            nv.reciprocal(rgw[:], g_w[:])
            delta = t3("delta")
            nv.tensor_tensor(delta[:], g_h[:], rgw[:], Alu.mult)
            theta = t3("theta")
            nv.tensor_tensor(theta[:], xin[:], g_cw[:], Alu.subtract)
            nv.tensor_tensor(theta[:], theta[:], rgw[:], Alu.mult)
            omt = t3("omt")
            nv.tensor_scalar(omt[:], theta[:], -1.0, 1.0, Alu.mult, Alu.add)
            tomt = t3("tomt")
            nv.tensor_tensor(tomt[:], theta[:], omt[:], Alu.mult)
            th2 = t3("th2")
            nv.tensor_tensor(th2[:], theta[:], theta[:], Alu.mult)
            omt2 = t3("omt2")
            nv.tensor_tensor(omt2[:], omt[:], omt[:], Alu.mult)

            # num = g_h * (delta*th2 + g_d*tomt)
            t_a = t3("t_a")
            nv.tensor_tensor(t_a[:], delta[:], th2[:], Alu.mult)
            t_b = t3("t_b")
            nv.tensor_tensor(t_b[:], g_d[:], tomt[:], Alu.mult)
            nv.tensor_tensor(t_a[:], t_a[:], t_b[:], Alu.add)
            num = t3("num")
            nv.tensor_tensor(num[:], g_h[:], t_a[:], Alu.mult)
            # den = delta + (g_d + g_d1 - 2*delta) * tomt
            t_c = t3("t_c")
            nv.tensor_tensor(t_c[:], g_d[:], g_d1[:], Alu.add)
            nv.scalar_tensor_tensor(t_c[:], delta[:], -2.0, t_c[:],
                                    Alu.mult, Alu.add)
            nv.tensor_tensor(t_c[:], t_c[:], tomt[:], Alu.mult)
            den = t3("den")
            nv.tensor_tensor(den[:], t_c[:], delta[:], Alu.add)
            rden = t3("rden")
            nv.reciprocal(rden[:], den[:])
            outv = t3("outv")
            nv.tensor_tensor(outv[:], num[:], rden[:], Alu.mult)
            nv.tensor_tensor(outv[:], outv[:], g_ch[:], Alu.add)

            # dnum = delta^2*(g_d1*th2 + 2*delta*tomt + g_d*omt2)
            t_d = t3("t_d")
            nv.tensor_tensor(t_d[:], g_d1[:], th2[:], Alu.mult)
            t_e = t3("t_e")
            nv.scalar_tensor_tensor(t_e[:], delta[:], 2.0, tomt[:],
                                    Alu.mult, Alu.mult)
            nv.tensor_tensor(t_d[:], t_d[:], t_e[:], Alu.add)
            t_f = t3("t_f")
            nv.tensor_tensor(t_f[:], g_d[:], omt2[:], Alu.mult)
            nv.tensor_tensor(t_d[:], t_d[:], t_f[:], Alu.add)
            d2 = t3("d2")
            nv.tensor_tensor(d2[:], delta[:], delta[:], Alu.mult)
            nv.tensor_tensor(t_d[:], t_d[:], d2[:], Alu.mult)
            # lad = ln(dnum * rden^2)
            nv.tensor_tensor(t_d[:], t_d[:], rden[:], Alu.mult)
            nv.tensor_tensor(t_d[:], t_d[:], rden[:], Alu.mult)
            lad = t3("lad")
            ns.activation(lad[:], t_d[:], Act.Ln)

            # --- inside mask + select ---
            ins1 = t3("ins1")
            nv.tensor_scalar(ins1[:], x1m, -TB, None, Alu.is_ge)
            ins2 = t3("ins2")
            nv.tensor_scalar(ins2[:], x1m, TB, None, Alu.is_le)
            inside = t3("inside")
            nv.tensor_tensor(inside[:], ins1[:], ins2[:], Alu.mult)

            inside_i = sp.tile([128, NTT, HALF], dt.int32, tag="inside_i",
                               name=f"sp_inside_i_{e}")
            nv.tensor_copy(inside_i[:], inside[:])
            x1n = t3("x1n")
            nv.tensor_copy(x1n[:], x1m)
            nv.copy_predicated(x1n[:], inside_i[:], outv[:])

            # masked outputs
            if trivial_mask:
                x0m = x_mega[e][:, :, 0:HALF]
            else:
                mgb = mask_mg[e][:].unsqueeze(2).broadcast_to(
                    (128, NTT, HALF))
                x0m_t = t3("x0m")
                nv.tensor_tensor(x0m_t[:], x_mega[e][:, :, 0:HALF], mgb,
                                 Alu.mult)
                nv.tensor_tensor(x1n[:], x1n[:], mgb, Alu.mult)
                x0m = x0m_t[:]

            out_r = out_sh[e].rearrange("c (j p) -> p j c", p=128)
            for hc in range(HALF):
                nc.sync.dma_start(out_r[:, :, hc:hc + 1],
                                  x0m[:, :, hc:hc + 1])
                nc.sync.dma_start(out_r[:, :, HALF + hc:HALF + hc + 1],
                                  x1n[:, :, hc:hc + 1])

            # --- logdet ---
            ladm = t3("ladm")
            nv.tensor_tensor(ladm[:], lad[:], inside[:], Alu.mult)
            if not trivial_mask:
                nv.tensor_tensor(ladm[:], ladm[:], mgb, Alu.mult)
            ldp = sp.tile([128, 1], dt.float32, tag="ldp", name=f"ldp{e}")
            nv.tensor_reduce(ldp[:], ladm[:], Ax.XY, Alu.add)
            pl = psum_proj.tile([1, 1], dt.float32, tag="pl", bufs=1,
                                name=f"pl{e}")
            nt.matmul(pl[:], ldp[:], ones_f[:])
            lds = sp.tile([1, 1], dt.float32, tag="lds", name=f"lds{e}")
            ns.copy(lds[:], pl[:])
            nc.sync.dma_start(ld_sh[e], lds[:])

    nc.compile()
    return nc


_prog_cache = {}


def _triviality(inputs):
    """Host-visible input properties the program specializes on."""
    tb = (not np.any(np.asarray(inputs["pre_b"]))
          and not np.any(np.asarray(inputs["conv_b"]))
          and not np.any(np.asarray(inputs["proj_b"])))
    tm = bool(np.all(np.asarray(inputs["x_mask"]) == 1.0))
    return tb, tm


def _get_program(trivial_bias=True, trivial_mask=True):
    key = (trivial_bias, trivial_mask)
    if key not in _prog_cache:
        _prog_cache[key] = build_program(trivial_bias, trivial_mask)
    return _prog_cache[key]


def make_in_maps(x, x_mask, pre_w, pre_b, conv_w, conv_b, ln_g, ln_b,
                 proj_w, proj_b):
    x = np.asarray(x, np.float32)
    x_mask = np.asarray(x_mask, np.float32)
    prew = np.asarray(pre_w, np.float32).reshape(F, HALF).T.astype(BF16)
    preb = np.asarray(pre_b, np.float32).reshape(NCG, 128).T.copy()
    convw = np.transpose(np.asarray(conv_w, np.float32),
                         (0, 3, 2, 1)).astype(BF16).copy()
    convb = np.asarray(conv_b, np.float32).astype(BF16)
    lng = np.asarray(ln_g, np.float32).reshape(NL, NCG, 128).transpose(
        0, 2, 1).copy()
    lnb = np.asarray(ln_b, np.float32).reshape(NL, NCG, 128).transpose(
        0, 2, 1).copy()
    projw = np.asarray(proj_w, np.float32).reshape(HALF * NP3, F).T.astype(
        BF16).copy()
    projb = np.tile(np.asarray(proj_b, np.float32)[None, :], (128, 1)).copy()
    ident = np.eye(128, dtype=BF16)
    iota = np.tile(np.arange(NB, dtype=np.float32)[None, :], (128, 1)).copy()
    onesc = np.ones((1, 128), BF16)
    onesf = np.ones((128, 1), np.float32)

    in_maps = []
    for c in range(NCORES):
        sl = slice(c * BPC, (c + 1) * BPC)
        xs = x[sl]
        ms = x_mask[sl]
        in_maps.append({
            "x_sh": xs.copy(),
            "x0_bf": xs[:, :HALF, :].astype(BF16).copy(),
            "maskbc": np.tile(ms[:, 0:1, :].astype(BF16), (1, 128, 1)).copy(),
            "maskmg": np.ascontiguousarray(
                ms[:, 0, :].reshape(BPC, NTT, 128).transpose(0, 2, 1)),
            "prew": prew.copy(), "preb": preb, "convw": convw,
            "convb": convb, "lng": lng, "lnb": lnb,
            "projw": projw, "projb": projb, "ident": ident,
            "iota10": iota, "onesc": onesc, "onesf": onesf,
        })
    return in_maps


def kernel(**inputs):
    from concourse.bass_utils import run_bass_kernel_spmd
    tb, tm = _triviality(inputs)
    nc = _get_program(tb, tm)
    in_maps = make_in_maps(**inputs)
    res = run_bass_kernel_spmd(nc, in_maps, list(range(NCORES))).results
    out = np.concatenate([r["out_sh"] for r in res], axis=0)
    logdet = np.concatenate([r["ld_sh"].reshape(BPC) for r in res], axis=0)
    return out.astype(np.float32), logdet.astype(np.float32)


# revision 33
# speedup vs baseline: 1.2098x; 1.0772x over previous
"""ConvFlow (VITS-style coupling layer) Trainium2 kernel.

Data-parallel over 8 NeuronCores: 2 batch examples per core. Per core:
  x0 -> 1x1 pre-conv -> 4x [conv1d(k=5) -> LayerNorm(C) -> GELU] -> 1x1 proj
     -> rational-quadratic spline applied to x1; outputs (out, logdet).

Layout strategy:
  - Activations h live in SBUF as bf16 [cin(=128-part) x T(+4 pad)] tiles,
    4 channel-groups x 2 examples, ping-pong buffers between layers.
  - conv: psum[t(128), cout(512)] += sum_{cg,k} h[cg, t+k-2].T @ wT[k][cg]
    (+ K=1 ones-row matmul for the bias). LN stats on the psum tile
    (free-dim = channels), xhat=(x-mu)*rstd via one tensor_scalar -> bf16,
    PE-transpose 128x128 blocks back to [c, t], ACT gelu(g*x + b) with
    per-partition ln params writes the next h tile.
  - spline params in "mega" layout [128(t%128) x 32(t//128) x 2(half) x K]
    so every elementwise/bin op is a wide DVE/ACT op.
"""

import math
import numpy as np
import ml_dtypes

B, C_IN, T = 16, 4, 4096
HALF = C_IN // 2
F = 512
KS = 5
NL = 4
NB = 10            # NUM_BINS
TB = 5.0           # tail bound
MIN_BW = 1e-3
MIN_BH = 1e-3
MIN_D = 1e-3
EPS = 1e-5
NCORES = 8
BPC = B // NCORES  # examples per core = 2
NCG = F // 128     # channel groups = 4
NTT = T // 128     # token tiles per example = 32
NTB = T // 512     # 512-wide t banks = 8
TPAD = T + 4       # padded free dim for conv halo
SSCALE = 1.0 / math.sqrt(F)
NP3 = 3 * NB - 1   # 29 params per half
BF16 = ml_dtypes.bfloat16


def build_program(trivial_bias=True, trivial_mask=True):
    import concourse.bacc as bacc
    import concourse.tile as tile
    from concourse import mybir
    from contextlib import ExitStack

    dt = mybir.dt
    Alu = mybir.AluOpType
    Act = mybir.ActivationFunctionType
    Ax = mybir.AxisListType

    nc = bacc.Bacc("TRN2", target_bir_lowering=False, debug=False,
                   num_devices=NCORES)

    # ---- DRAM I/O ----
    def din(name, shape, dtype):
        return nc.dram_tensor(name, list(shape), dtype, kind="ExternalInput").ap()

    x_sh = din("x_sh", (BPC, C_IN, T), dt.float32)
    x0_bf = din("x0_bf", (BPC, HALF, T), dt.bfloat16)
    maskbc = din("maskbc", (BPC, 128, T), dt.bfloat16)
    maskmg = din("maskmg", (BPC, 128, NTT), dt.float32)
    prew_d = din("prew", (HALF, F), dt.bfloat16)
    preb_d = din("preb", (128, NCG), dt.float32)
    convw_d = din("convw", (NL, KS, F, F), dt.bfloat16)
    convb_d = din("convb", (NL, F), dt.bfloat16)
    lng_d = din("lng", (NL, 128, NCG), dt.float32)
    lnb_d = din("lnb", (NL, 128, NCG), dt.float32)
    projw_d = din("projw", (F, HALF * NP3), dt.bfloat16)
    projb_d = din("projb", (128, HALF * NP3), dt.float32)
    ident_d = din("ident", (128, 128), dt.bfloat16)
    iota_d = din("iota10", (128, NB), dt.float32)
    onesc_d = din("onesc", (1, 128), dt.bfloat16)
    onesf_d = din("onesf", (128, 1), dt.float32)

    out_sh = nc.dram_tensor("out_sh", [BPC, C_IN, T], dt.float32,
                            kind="ExternalOutput").ap()
    ld_sh = nc.dram_tensor("ld_sh", [BPC, 1], dt.float32,
                           kind="ExternalOutput").ap()

    with tile.TileContext(nc) as tc, ExitStack() as ctx:
        nv, ns, nt = nc.vector, nc.scalar, nc.tensor

        # ---------- constant / persistent pools ----------
        consts = ctx.enter_context(tc.tile_pool(name="consts", bufs=1))
        ident = consts.tile([128, 128], dt.bfloat16)
        nc.sync.dma_start(ident[:], ident_d[:])
        iota10 = consts.tile([128, NB], dt.float32)
        nc.sync.dma_start(iota10[:], iota_d[:])
        ones_col = consts.tile([1, 128], dt.bfloat16)
        nc.sync.dma_start(ones_col[:], onesc_d[:])
        ones_f = consts.tile([128, 1], dt.float32)
        nc.sync.dma_start(ones_f[:], onesf_d[:])
        eps_t = consts.tile([128, 1], dt.float32)
        nv.memset(eps_t[:], EPS)
        prew = consts.tile([HALF, F], dt.bfloat16)
        nc.sync.dma_start(prew[:], prew_d[:])
        preb = consts.tile([128, NCG], dt.float32)
        nc.sync.dma_start(preb[:], preb_d[:])
        convb = consts.tile([1, NL, F], dt.bfloat16)
        nc.sync.dma_start(convb[:], convb_d.unsqueeze(0))
        lng = consts.tile([128, NL, NCG], dt.float32)
        nc.sync.dma_start(lng[:], lng_d.rearrange("l p c -> p l c"))
        lnb = consts.tile([128, NL, NCG], dt.float32)
        nc.sync.dma_start(lnb[:], lnb_d.rearrange("l p c -> p l c"))
        projw = consts.tile([128, NCG, HALF * NP3], dt.bfloat16)
        nc.sync.dma_start(projw[:], projw_d.rearrange("(g p) n -> p g n", p=128))
        projb = consts.tile([128, HALF * NP3], dt.float32)
        nc.sync.dma_start(projb[:], projb_d[:])

        mask_bc = []
        mask_mg = []
        x_mega = []
        for e in range(BPC):
            if not trivial_mask:
                mb = consts.tile([128, T], dt.bfloat16, name=f"mask_bc{e}")
                nc.sync.dma_start(mb[:], maskbc[e])
                mask_bc.append(mb)
                mm = consts.tile([128, NTT], dt.float32, name=f"mask_mg{e}")
                nc.sync.dma_start(mm[:], maskmg[e])
                mask_mg.append(mm)
            xm = consts.tile([128, NTT, C_IN], dt.float32, name=f"x_mega{e}")
            for c in range(C_IN):
                nc.sync.dma_start(
                    xm[:, :, c:c + 1],
                    x_sh[e, c].rearrange("(j p) -> p j", p=128).unsqueeze(2))
            x_mega.append(xm)

        # spline params per example, [128, NTT, 58] f32
        params_pool = ctx.enter_context(tc.tile_pool(name="params", bufs=1))
        params = [params_pool.tile([128, NTT, HALF * NP3], dt.float32,
                                   name=f"params{e}") for e in range(BPC)]

        # ---------- psum + small work pools (live whole kernel) ----------
        psum_conv = ctx.enter_context(
            tc.tile_pool(name="psum_conv", bufs=2, space="PSUM"))
        psum_tp = ctx.enter_context(
            tc.tile_pool(name="psum_tp", bufs=3, space="PSUM"))
        psum_proj = ctx.enter_context(
            tc.tile_pool(name="psum_proj", bufs=2, space="PSUM"))
        stats = ctx.enter_context(tc.tile_pool(name="stats", bufs=2))
        xhat_pool = ctx.enter_context(
            tc.tile_pool(name="xhat", bufs=2 if trivial_mask else 1))

        def conv_block(psum, h_src, e, j, wt, bias_row):
            """psum[t,cout] = sum_{cg,k} h.T @ w (+ ones.T @ bias)"""
            nmm = NCG * KS
            i = 0
            for cg in range(NCG):
                for k in range(KS):
                    lhsT = h_src[e][cg][:, j * 128 + k: j * 128 + k + 128]
                    nt.matmul(psum[:], lhsT, wt[cg][k][:], start=(i == 0),
                              stop=(trivial_bias and i == nmm - 1))
                    i += 1
            if not trivial_bias:
                nt.matmul(psum[:], ones_col[:], bias_row,
                          start=False, stop=True)

        def ln_chunk(h_dst, e, j):
            """Scratch slot for tile j's raw conv output (token-major),
            parked inside the destination buffer: chunk m=j//4 of tile
            cg=j%4. gelu writes of tile j only overwrite chunk j//4, whose
            four parked tiles (quad 4*(j//4)..+3) are consumed first."""
            m = j // 4
            return h_dst[e][j % 4][:, 2 + 512 * m: 2 + 512 * (m + 1)]

        def ph1_units(e, l, src, dst, wt):
            """32 emission units: conv tile -> park chunk + sum/sumsq (DVE)."""
            sb = stats.tile([128, NTT], dt.float32, tag="sumb",
                            name=f"sumb{l}_{e}")
            sqb = stats.tile([128, NTT], dt.float32, tag="sqb",
                             name=f"sqb{l}_{e}")

            def unit(j):
                ps = psum_conv.tile([128, F], dt.float32, tag="ps",
                                    name=f"ps{l}_{e}_{j}")
                conv_block(ps, src, e, j, wt, convb[:, l, :])
                nv.tensor_scalar(ln_chunk(dst, e, j), ps[:], 0.0, None,
                                 Alu.add, Alu.add,
                                 accum_out=sb[:, j:j + 1])
                sqd = stats.tile([128, F], dt.bfloat16, tag="sqd",
                                 name=f"sqd{l}_{e}_{j}")
                ch = ln_chunk(dst, e, j)
                nv.scalar_tensor_tensor(sqd[:], ch, 1.0, ch,
                                        Alu.mult, Alu.mult,
                                        accum_out=sqb[:, j:j + 1])

            return [lambda j=j: unit(j) for j in range(NTT)], (sb, sqb)

        def ph23_units(e, l, dst, sb, sqb):
            """33 units: batched rstd prelude, then (quad,cg) tp+gelu."""
            state = {}

            def prelude():
                meanb = stats.tile([128, NTT], dt.float32, tag="meanb",
                                   name=f"meanb{l}_{e}")
                nv.tensor_scalar(meanb[:], sb[:], 1.0 / F, None, Alu.mult)
                varb = stats.tile([128, NTT], dt.float32, tag="varb",
                                  name=f"varb{l}_{e}")
                nv.scalar_tensor_tensor(varb[:], meanb[:], 0.0, meanb[:],
                                        Alu.add, Alu.mult)
                nv.scalar_tensor_tensor(varb[:], sqb[:], 1.0 / F, varb[:],
                                        Alu.mult, Alu.subtract)
                stdb = stats.tile([128, NTT], dt.float32, tag="stdb",
                                  name=f"stdb{l}_{e}")
                ns.activation(stdb[:], varb[:], Act.Sqrt, bias=eps_t[:])
                rstdb = stats.tile([128, NTT], dt.float32, tag="rstdb",
                                   name=f"rstdb{l}_{e}")
                nv.reciprocal(rstdb[:], stdb[:])
                state["m"] = meanb
                state["r"] = rstdb

            def unit(q, cg):
                if cg == 0:
                    state["xh"] = []
                    for jj in range(4):
                        j = 4 * q + jj
                        x = xhat_pool.tile([128, F], dt.bfloat16,
                                           tag=f"xh{jj}",
                                           name=f"xh{l}_{e}_{j}")
                        nv.tensor_scalar(x[:], ln_chunk(dst, e, j),
                                         state["m"][:, j:j + 1],
                                         state["r"][:, j:j + 1],
                                         Alu.subtract, Alu.mult)
                        state["xh"].append(x)
                xh = state["xh"]
                ptb = psum_tp.tile([128, 512], dt.bfloat16, tag="pt",
                                   name=f"pt{l}_{e}_{q}_{cg}")
                for jj in range(4):
                    nt.matmul(ptb[:, jj * 128:(jj + 1) * 128],
                              xh[jj][:, cg * 128:(cg + 1) * 128],
                              ident[:], is_transpose=True)
                ns.activation(
                    dst[e][cg][:, 2 + 512 * q: 2 + 512 * (q + 1)],
                    ptb[:], Act.Gelu,
                    bias=lnb[:, l, cg:cg + 1],
                    scale=lng[:, l, cg:cg + 1])

            return [prelude] + [lambda q=q, cg=cg: unit(q, cg)
                                for q in range(NTT // 4)
                                for cg in range(NCG)]

        def run_zip(a_units, b_units):
            """Interleave two unit lists (b may have a +1 prelude)."""
            if len(b_units) == len(a_units) + 1:
                b_units[0]()
                b_units = b_units[1:]
            n = max(len(a_units), len(b_units))
            for i in range(n):
                if i < len(a_units):
                    a_units[i]()
                if i < len(b_units):
                    b_units[i]()

        # ---------- activation ping-pong + weight pools ----------
        with tc.tile_pool(name="h_a", bufs=1) as h_a_pool:
            h_a = [[h_a_pool.tile([128, TPAD], dt.bfloat16, name=f"ha{e}_{cg}")
                    for cg in range(NCG)] for e in range(BPC)]
            with tc.tile_pool(name="h_b", bufs=1) as h_b_pool, \
                 tc.tile_pool(name="wpool", bufs=2) as wpool:
                h_b = [[h_b_pool.tile([128, TPAD], dt.bfloat16,
                                      name=f"hb{e}_{cg}")
                        for cg in range(NCG)] for e in range(BPC)]
                # zero the 2-col halos once; interiors are fully overwritten
                for hs in (h_a, h_b):
                    for e in range(BPC):
                        for cg in range(NCG):
                            nv.memset(hs[e][cg][:, 0:2], 0.0)
                            nv.memset(hs[e][cg][:, T + 2:T + 4], 0.0)

                def load_w(l):
                    wt = []
                    for cg in range(NCG):
                        row = []
                        for k in range(KS):
                            wb = 2 if trivial_mask else (
                                2 if (cg < 3 and k < 4) else 1)
                            w = wpool.tile([128, F], dt.bfloat16,
                                           tag=f"w{cg}_{k}", bufs=wb,
                                           name=f"w{l}_{cg}_{k}")
                            nc.sync.dma_start(
                                w[:],
                                convw_d[l, k, cg * 128:(cg + 1) * 128, :])
                            row.append(w)
                        wt.append(row)
                    return wt

                # ----- pre conv (1x1, K=2): x0 -> h_a, + bias, * mask -----
                wt_next = load_w(0)
                with tc.tile_pool(name="x0pool",
                                  bufs=2 if trivial_mask else 1) as x0pool:
                    for e in range(BPC):
                        for jb in range(NTB):
                            x0c = x0pool.tile([HALF, 512], dt.bfloat16,
                                              tag="x0c", name=f"x0c{e}_{jb}")
                            nc.sync.dma_start(
                                x0c[:], x0_bf[e][:, jb * 512:(jb + 1) * 512])
                            for cg in range(NCG):
                                ps = psum_conv.tile([128, 512], dt.float32,
                                                    tag="ps")
                                nt.matmul(ps[:],
                                          prew[:, cg * 128:(cg + 1) * 128],
                                          x0c[:])
                                hdst = h_a[e][cg][:, 2 + jb * 512:
                                                  2 + (jb + 1) * 512]
                                if trivial_bias and trivial_mask:
                                    ns.copy(hdst, ps[:])
                                elif trivial_mask:
                                    nv.tensor_scalar(hdst, ps[:],
                                                     preb[:, cg:cg + 1], None,
                                                     Alu.add)
                                else:
                                    nv.scalar_tensor_tensor(
                                        hdst, ps[:], preb[:, cg:cg + 1],
                                        mask_bc[e][:, jb * 512:(jb + 1) * 512],
                                        Alu.add, Alu.mult)

                # ----- 4 conv layers + proj, software-pipelined -----
                def proj_unit(e, j, h4):
                    pp = psum_proj.tile([128, HALF * NP3], dt.float32,
                                        tag="pp", name=f"pp{e}_{j}")
                    for cg in range(NCG):
                        nt.matmul(
                            pp[:],
                            h4[e][cg][:, 2 + j * 128: 2 + (j + 1) * 128],
                            projw[:, cg, :],
                            start=(cg == 0), stop=(cg == NCG - 1))
                    if trivial_bias and trivial_mask:
                        nv.tensor_copy(params[e][:, j, :], pp[:])
                    elif trivial_mask:
                        nv.tensor_tensor(params[e][:, j, :], pp[:],
                                         projb[:], Alu.add)
                    else:
                        tmp = stats.tile([128, HALF * NP3], dt.float32,
                                         tag="pj", name=f"pj{e}_{j}")
                        nv.tensor_tensor(tmp[:], pp[:], projb[:], Alu.add)
                        nv.tensor_scalar(params[e][:, j, :], tmp[:],
                                         mask_mg[e][:, j:j + 1], None,
                                         Alu.mult)

                wt_cur = wt_next
                prev_w = None
                src, dst = h_a, h_b
                for l in range(NL):
                    for e in range(BPC):
                        z, stt = ph1_units(e, l, src, dst, wt_cur)
                        if prev_w is None:
                            for u in z:
                                u()
                        else:
                            run_zip(z, prev_w)
                        prev_w = ph23_units(e, l, dst, *stt)
                        if e == 0 and l + 1 < NL:
                            wt_next = load_w(l + 1)
                        if (not trivial_mask) and e == BPC - 1 \
                                and l < NL - 1:
                            # general path: drain pipeline, apply mask
                            for u in prev_w:
                                u()
                            prev_w = None
                            for ee in range(BPC):
                                for cg in range(NCG):
                                    nv.tensor_tensor(
                                        dst[ee][cg][:, 2:2 + T],
                                        dst[ee][cg][:, 2:2 + T],
                                        mask_bc[ee][:], Alu.mult)
                    wt_cur = wt_next
                    src, dst = dst, src
                # final h (gelu4 out, unmasked) now in `src`
                h4 = src
                pu0 = [lambda j=j: proj_unit(0, j, h4) for j in range(NTT)]
                if prev_w is not None:
                    run_zip(pu0, prev_w)
                else:
                    for u in pu0:
                        u()
                for j in range(NTT):
                    proj_unit(1, j, h4)
            # h_b + wpool released; h_a no longer needed either

        # h_a released here
        # ---------- spline phase ----------
        sp = ctx.enter_context(tc.tile_pool(name="spline", bufs=2))

        for e in range(BPC):
            P4 = params[e][:].rearrange("p j (h k) -> p j h k", h=HALF)
            x1m = x_mega[e][:, :, HALF:C_IN]            # [128, 32, 2]

            def t3(tag, k=1):
                if k == 1:
                    return sp.tile([128, NTT, HALF], dt.float32, tag=tag,
                                   name=f"sp_{tag}_{e}")
                return sp.tile([128, NTT, HALF, k], dt.float32, tag=tag,
                               name=f"sp_{tag}_{e}")

            def bcast_in(a):
                """[128,32,2] -> broadcast over innermost K dim."""
                return a.unsqueeze(3)

            # --- softmax(w), softmax(h) (no max-sub; inputs are small) ---
            ew = t3("ew", NB)
            ns.activation(ew[:], P4[:, :, :, 0:NB], Act.Exp, scale=SSCALE)
            eh = t3("eh", NB)
            ns.activation(eh[:], P4[:, :, :, NB:2 * NB], Act.Exp, scale=SSCALE)
            sw = t3("sw")
            nv.tensor_reduce(sw[:], ew[:], Ax.X, Alu.add)
            sh = t3("sh")
            nv.tensor_reduce(sh[:], eh[:], Ax.X, Alu.add)
            rw = t3("rw")
            nv.reciprocal(rw[:], sw[:])
            rh = t3("rh")
            nv.reciprocal(rh[:], sh[:])

            # cumsum buffers [.., 18]: cols 8..17 hold data
            ca = t3("ca", 18)
            cb = t3("cb", 18)
            cc = t3("cc", 18)
            cd = t3("cd", 18)

            def norm_cumsum(ebuf, rbuf, a, b, minb):
                # a[..,8:18] = minb + (1-minb*NB) * e * r
                nv.scalar_tensor_tensor(a[:, :, :, 8:18], ebuf[:],
                                        0.0, rbuf.unsqueeze(3).broadcast_to((128, NTT, HALF, NB)),
                                        Alu.add, Alu.mult)
                nv.tensor_scalar(a[:, :, :, 8:18], a[:, :, :, 8:18],
                                 1.0 - minb * NB, minb, Alu.mult, Alu.add)
                nv.memset(a[:, :, :, 0:8], 0.0)
                nv.memset(b[:, :, :, 0:8], 0.0)
                nv.tensor_tensor(b[:, :, :, 8:18], a[:, :, :, 8:18],
                                 a[:, :, :, 7:17], Alu.add)
                nv.tensor_tensor(a[:, :, :, 8:18], b[:, :, :, 8:18],
                                 b[:, :, :, 6:16], Alu.add)
                nv.tensor_tensor(b[:, :, :, 8:18], a[:, :, :, 8:18],
                                 a[:, :, :, 4:14], Alu.add)
                nv.tensor_tensor(a[:, :, :, 8:18], b[:, :, :, 8:18],
                                 b[:, :, :, 0:10], Alu.add)
                # a[..,8:17] = cumsum_1..9 ; build knots
                return a

            cwsum = norm_cumsum(ew, rw, ca, cb, MIN_BW)
            chsum = norm_cumsum(eh, rh, cc, cd, MIN_BH)

            cw = t3("cw", NB + 1)
            nv.tensor_scalar(cw[:, :, :, 1:NB], cwsum[:, :, :, 8:17],
                             2.0 * TB, -TB, Alu.mult, Alu.add)
            nv.memset(cw[:, :, :, 0:1], -TB)
            nv.memset(cw[:, :, :, NB:NB + 1], TB)
            ch = t3("ch", NB + 1)
            nv.tensor_scalar(ch[:, :, :, 1:NB], chsum[:, :, :, 8:17],
                             2.0 * TB, -TB, Alu.mult, Alu.add)
            nv.memset(ch[:, :, :, 0:1], -TB)
            nv.memset(ch[:, :, :, NB:NB + 1], TB)

            wbin = t3("wbin", NB)
            nv.tensor_tensor(wbin[:], cw[:, :, :, 1:NB + 1],
                             cw[:, :, :, 0:NB], Alu.subtract)
            hbin = t3("hbin", NB)
            nv.tensor_tensor(hbin[:], ch[:, :, :, 1:NB + 1],
                             ch[:, :, :, 0:NB], Alu.subtract)

            # --- derivatives d[0..10]: ends exactly 1.0 ---
            dd = t3("dd", NB + 1)
            ns.activation(dd[:, :, :, 1:NB], P4[:, :, :, 2 * NB:NP3],
                          Act.Exp)
            nv.tensor_scalar(dd[:, :, :, 1:NB], dd[:, :, :, 1:NB],
                             1.0, None, Alu.add)
            ns.activation(dd[:, :, :, 1:NB], dd[:, :, :, 1:NB], Act.Ln)
            nv.tensor_scalar(dd[:, :, :, 1:NB], dd[:, :, :, 1:NB],
                             MIN_D, None, Alu.add)
            nv.memset(dd[:, :, :, 0:1], 1.0)
            nv.memset(dd[:, :, :, NB:NB + 1], 1.0)

            # --- bin index ---
            xin = t3("xin")
            nv.tensor_scalar(xin[:], x1m, -TB, TB, Alu.max, Alu.min)
            ge = t3("ge", NB)
            nv.tensor_tensor(ge[:], bcast_in(xin[:]).broadcast_to(
                (128, NTT, HALF, NB)), cw[:, :, :, 0:NB], Alu.is_ge)
            idx = t3("idx")
            nv.tensor_reduce(idx[:], ge[:], Ax.X, Alu.add)
            nv.tensor_scalar(idx[:], idx[:], -1.0, None, Alu.add)
            oh = t3("oh", NB)
            nv.tensor_tensor(
                oh[:],
                iota10[:].unsqueeze(1).unsqueeze(1).broadcast_to(
                    (128, NTT, HALF, NB)),
                bcast_in(idx[:]).broadcast_to((128, NTT, HALF, NB)),
                Alu.is_equal)

            # --- gathers via one-hot ---
            def gather(src_ap, tag):
                t = t3("gt_" + tag, NB)
                nv.tensor_tensor(t[:], src_ap, oh[:], Alu.mult)
                g = t3("g_" + tag)
                nv.tensor_reduce(g[:], t[:], Ax.X, Alu.add)
                return g

            g_cw = gather(cw[:, :, :, 0:NB], "cw")
            g_w = gather(wbin[:], "w")
            g_ch = gather(ch[:, :, :, 0:NB], "ch")
            g_h = gather(hbin[:], "h")
            g_d = gather(dd[:, :, :, 0:NB], "d")
            g_d1 = gather(dd[:, :, :, 1:NB + 1], "d1")

            # --- rational quadratic ---
            rgw = t3("rgw")
            nv.reciprocal(rgw[:], g_w[:])
            delta = t3("delta")
            nv.tensor_tensor(delta[:], g_h[:], rgw[:], Alu.mult)
            theta = t3("theta")
            nv.tensor_tensor(theta[:], xin[:], g_cw[:], Alu.subtract)
            nv.tensor_tensor(theta[:], theta[:], rgw[:], Alu.mult)
            omt = t3("omt")
            nv.tensor_scalar(omt[:], theta[:], -1.0, 1.0, Alu.mult, Alu.add)
            tomt = t3("tomt")
            nv.tensor_tensor(tomt[:], theta[:], omt[:], Alu.mult)
            th2 = t3("th2")
            nv.tensor_tensor(th2[:], theta[:], theta[:], Alu.mult)
            omt2 = t3("omt2")
            nv.tensor_tensor(omt2[:], omt[:], omt[:], Alu.mult)

            # num = g_h * (delta*th2 + g_d*tomt)
            t_a = t3("t_a")
            nv.tensor_tensor(t_a[:], delta[:], th2[:], Alu.mult)
            t_b = t3("t_b")
            nv.tensor_tensor(t_b[:], g_d[:], tomt[:], Alu.mult)
            nv.tensor_tensor(t_a[:], t_a[:], t_b[:], Alu.add)
            num = t3("num")
            nv.tensor_tensor(num[:], g_h[:], t_a[:], Alu.mult)
            # den = delta + (g_d + g_d1 - 2*delta) * tomt
            t_c = t3("t_c")
            nv.tensor_tensor(t_c[:], g_d[:], g_d1[:], Alu.add)
            nv.scalar_tensor_tensor(t_c[:], delta[:], -2.0, t_c[:],
                                    Alu.mult, Alu.add)
            nv.tensor_tensor(t_c[:], t_c[:], tomt[:], Alu.mult)
            den = t3("den")
            nv.tensor_tensor(den[:], t_c[:], delta[:], Alu.add)
            rden = t3("rden")
            nv.reciprocal(rden[:], den[:])
            outv = t3("outv")
            nv.tensor_tensor(outv[:], num[:], rden[:], Alu.mult)
            nv.tensor_tensor(outv[:], outv[:], g_ch[:], Alu.add)

            # dnum = delta^2*(g_d1*th2 + 2*delta*tomt + g_d*omt2)
            t_d = t3("t_d")
            nv.tensor_tensor(t_d[:], g_d1[:], th2[:], Alu.mult)
            t_e = t3("t_e")
            nv.scalar_tensor_tensor(t_e[:], delta[:], 2.0, tomt[:],
                                    Alu.mult, Alu.mult)
            nv.tensor_tensor(t_d[:], t_d[:], t_e[:], Alu.add)
            t_f = t3("t_f")
            nv.tensor_tensor(t_f[:], g_d[:], omt2[:], Alu.mult)
            nv.tensor_tensor(t_d[:], t_d[:], t_f[:], Alu.add)
            d2 = t3("d2")
            nv.tensor_tensor(d2[:], delta[:], delta[:], Alu.mult)
            nv.tensor_tensor(t_d[:], t_d[:], d2[:], Alu.mult)
            # lad = ln(dnum * rden^2)
            nv.tensor_tensor(t_d[:], t_d[:], rden[:], Alu.mult)
            nv.tensor_tensor(t_d[:], t_d[:], rden[:], Alu.mult)
            lad = t3("lad")
            ns.activation(lad[:], t_d[:], Act.Ln)

            # --- inside mask + select ---
            ins1 = t3("ins1")
            nv.tensor_scalar(ins1[:], x1m, -TB, None, Alu.is_ge)
            ins2 = t3("ins2")
            nv.tensor_scalar(ins2[:], x1m, TB, None, Alu.is_le)
            inside = t3("inside")
            nv.tensor_tensor(inside[:], ins1[:], ins2[:], Alu.mult)

            inside_i = sp.tile([128, NTT, HALF], dt.int32, tag="inside_i",
                               name=f"sp_inside_i_{e}")
            nv.tensor_copy(inside_i[:], inside[:])
            x1n = t3("x1n")
            nv.tensor_copy(x1n[:], x1m)
            nv.copy_predicated(x1n[:], inside_i[:], outv[:])

            # masked outputs
            if trivial_mask:
                x0m = x_mega[e][:, :, 0:HALF]
            else:
                mgb = mask_mg[e][:].unsqueeze(2).broadcast_to(
                    (128, NTT, HALF))
                x0m_t = t3("x0m")
                nv.tensor_tensor(x0m_t[:], x_mega[e][:, :, 0:HALF], mgb,
                                 Alu.mult)
                nv.tensor_tensor(x1n[:], x1n[:], mgb, Alu.mult)
                x0m = x0m_t[:]

            out_r = out_sh[e].rearrange("c (j p) -> p j c", p=128)
            for hc in range(HALF):
                nc.sync.dma_start(out_r[:, :, hc:hc + 1],
                                  x0m[:, :, hc:hc + 1])
                nc.sync.dma_start(out_r[:, :, HALF + hc:HALF + hc + 1],
                                  x1n[:, :, hc:hc + 1])

            # --- logdet ---
            ladm = t3("ladm")
            nv.tensor_tensor(ladm[:], lad[:], inside[:], Alu.mult)
            if not trivial_mask:
                nv.tensor_tensor(ladm[:], ladm[:], mgb, Alu.mult)
            ldp = sp.tile([128, 1], dt.float32, tag="ldp", name=f"ldp{e}")
            nv.tensor_reduce(ldp[:], ladm[:], Ax.XY, Alu.add)
            pl = psum_proj.tile([1, 1], dt.float32, tag="pl", bufs=1,
                                name=f"pl{e}")
            nt.matmul(pl[:], ldp[:], ones_f[:])
            lds = sp.tile([1, 1], dt.float32, tag="lds", name=f"lds{e}")
            ns.copy(lds[:], pl[:])
            nc.sync.dma_start(ld_sh[e], lds[:])

    nc.compile()
    return nc


_prog_cache = {}


def _triviality(inputs):
    """Host-visible input properties the program specializes on."""
    tb = (not np.any(np.asarray(inputs["pre_b"]))
          and not np.any(np.asarray(inputs["conv_b"]))
          and not np.any(np.asarray(inputs["proj_b"])))
    tm = bool(np.all(np.asarray(inputs["x_mask"]) == 1.0))
    return tb, tm


def _get_program(trivial_bias=True, trivial_mask=True):
    key = (trivial_bias, trivial_mask)
    if key not in _prog_cache:
        _prog_cache[key] = build_program(trivial_bias, trivial_mask)
    return _prog_cache[key]


def make_in_maps(x, x_mask, pre_w, pre_b, conv_w, conv_b, ln_g, ln_b,
                 proj_w, proj_b):
    x = np.asarray(x, np.float32)
    x_mask = np.asarray(x_mask, np.float32)
    prew = np.asarray(pre_w, np.float32).reshape(F, HALF).T.astype(BF16)
    preb = np.asarray(pre_b, np.float32).reshape(NCG, 128).T.copy()
    convw = np.transpose(np.asarray(conv_w, np.float32),
                         (0, 3, 2, 1)).astype(BF16).copy()
    convb = np.asarray(conv_b, np.float32).astype(BF16)
    lng = np.asarray(ln_g, np.float32).reshape(NL, NCG, 128).transpose(
        0, 2, 1).copy()
    lnb = np.asarray(ln_b, np.float32).reshape(NL, NCG, 128).transpose(
        0, 2, 1).copy()
    projw = np.asarray(proj_w, np.float32).reshape(HALF * NP3, F).T.astype(
        BF16).copy()
    projb = np.tile(np.asarray(proj_b, np.float32)[None, :], (128, 1)).copy()
    ident = np.eye(128, dtype=BF16)
    iota = np.tile(np.arange(NB, dtype=np.float32)[None, :], (128, 1)).copy()
    onesc = np.ones((1, 128), BF16)
    onesf = np.ones((128, 1), np.float32)

    in_maps = []
    for c in range(NCORES):
        sl = slice(c * BPC, (c + 1) * BPC)
        xs = x[sl]
        ms = x_mask[sl]
        in_maps.append({
            "x_sh": xs.copy(),
            "x0_bf": xs[:, :HALF, :].astype(BF16).copy(),
            "maskbc": np.tile(ms[:, 0:1, :].astype(BF16), (1, 128, 1)).copy(),
            "maskmg": np.ascontiguousarray(
                ms[:, 0, :].reshape(BPC, NTT, 128).transpose(0, 2, 1)),
            "prew": prew.copy(), "preb": preb, "convw": convw,
            "convb": convb, "lng": lng, "lnb": lnb,
            "projw": projw, "projb": projb, "ident": ident,
            "iota10": iota, "onesc": onesc, "onesf": onesf,
        })
    return in_maps


def kernel(**inputs):
    from concourse.bass_utils import run_bass_kernel_spmd
    tb, tm = _triviality(inputs)
    nc = _get_program(tb, tm)
    in_maps = make_in_maps(**inputs)
    res = run_bass_kernel_spmd(nc, in_maps, list(range(NCORES))).results
    out = np.concatenate([r["out_sh"] for r in res], axis=0)
    logdet = np.concatenate([r["ld_sh"].reshape(BPC) for r in res], axis=0)
    return out.astype(np.float32), logdet.astype(np.float32)


# revision 35
# speedup vs baseline: 1.2313x; 1.0178x over previous
"""ConvFlow (VITS-style coupling layer) Trainium2 kernel.

Data-parallel over 8 NeuronCores: 2 batch examples per core. Per core:
  x0 -> 1x1 pre-conv -> 4x [conv1d(k=5) -> LayerNorm(C) -> GELU] -> 1x1 proj
     -> rational-quadratic spline applied to x1; outputs (out, logdet).

Layout strategy:
  - Activations h live in SBUF as bf16 [cin(=128-part) x T(+4 pad)] tiles,
    4 channel-groups x 2 examples, ping-pong buffers between layers.
  - conv: psum[t(128), cout(512)] += sum_{cg,k} h[cg, t+k-2].T @ wT[k][cg]
    (+ K=1 ones-row matmul for the bias). LN stats on the psum tile
    (free-dim = channels), xhat=(x-mu)*rstd via one tensor_scalar -> bf16,
    PE-transpose 128x128 blocks back to [c, t], ACT gelu(g*x + b) with
    per-partition ln params writes the next h tile.
  - spline params in "mega" layout [128(t%128) x 32(t//128) x 2(half) x K]
    so every elementwise/bin op is a wide DVE/ACT op.
"""

import math
import numpy as np
import ml_dtypes

B, C_IN, T = 16, 4, 4096
HALF = C_IN // 2
F = 512
KS = 5
NL = 4
NB = 10            # NUM_BINS
TB = 5.0           # tail bound
MIN_BW = 1e-3
MIN_BH = 1e-3
MIN_D = 1e-3
EPS = 1e-5
NCORES = 8
BPC = B // NCORES  # examples per core = 2
NCG = F // 128     # channel groups = 4
NTT = T // 128     # token tiles per example = 32
NTB = T // 512     # 512-wide t banks = 8
TPAD = T + 4       # padded free dim for conv halo
SSCALE = 1.0 / math.sqrt(F)
NP3 = 3 * NB - 1   # 29 params per half
BF16 = ml_dtypes.bfloat16


def build_program(trivial_bias=True, trivial_mask=True):
    import concourse.bacc as bacc
    import concourse.tile as tile
    from concourse import mybir
    from contextlib import ExitStack

    dt = mybir.dt
    Alu = mybir.AluOpType
    Act = mybir.ActivationFunctionType
    Ax = mybir.AxisListType

    nc = bacc.Bacc("TRN2", target_bir_lowering=False, debug=False,
                   num_devices=NCORES)

    # ---- DRAM I/O ----
    def din(name, shape, dtype):
        return nc.dram_tensor(name, list(shape), dtype, kind="ExternalInput").ap()

    x_mega_d = din("x_megah", (BPC, 128, NTT, C_IN), dt.float32)
    x0_bf = din("x0_bf", (BPC, HALF, T), dt.bfloat16)
    if not trivial_mask:
        maskbc = din("maskbc", (BPC, 128, T), dt.bfloat16)
        maskmg = din("maskmg", (BPC, 128, NTT), dt.float32)
    prew_d = din("prew", (HALF, F), dt.bfloat16)
    preb_d = din("preb", (128, NCG), dt.float32)
    convw_d = din("convw", (NL, KS, F, F), dt.bfloat16)
    convb_d = din("convb", (NL, F), dt.bfloat16)
    lng_d = din("lngh", (128, NL, NCG), dt.float32)
    lnb_d = din("lnbh", (128, NL, NCG), dt.float32)
    projw_d = din("projwh", (128, NCG, HALF * NP3), dt.bfloat16)
    projb_d = din("projb", (128, HALF * NP3), dt.float32)
    ident_d = din("ident", (128, 128), dt.bfloat16)
    iota_d = din("iota10", (128, NB), dt.float32)
    onesc_d = din("onesc", (1, 128), dt.bfloat16)
    onesf_d = din("onesf", (128, 1), dt.float32)

    out_sh = nc.dram_tensor("out_sh", [BPC, C_IN, T], dt.float32,
                            kind="ExternalOutput").ap()
    ld_sh = nc.dram_tensor("ld_sh", [BPC, 1], dt.float32,
                           kind="ExternalOutput").ap()

    with tile.TileContext(nc) as tc, ExitStack() as ctx:
        nv, ns, nt = nc.vector, nc.scalar, nc.tensor

        # ---------- constant / persistent pools ----------
        consts = ctx.enter_context(tc.tile_pool(name="consts", bufs=1))
        ident = consts.tile([128, 128], dt.bfloat16)
        nc.sync.dma_start(ident[:], ident_d[:])
        iota10 = consts.tile([128, NB], dt.float32)
        nc.sync.dma_start(iota10[:], iota_d[:])
        ones_col = consts.tile([1, 128], dt.bfloat16)
        nc.sync.dma_start(ones_col[:], onesc_d[:])
        ones_f = consts.tile([128, 1], dt.float32)
        nc.sync.dma_start(ones_f[:], onesf_d[:])
        eps_t = consts.tile([128, 1], dt.float32)
        nv.memset(eps_t[:], EPS)
        prew = consts.tile([HALF, F], dt.bfloat16)
        nc.sync.dma_start(prew[:], prew_d[:])
        preb = consts.tile([128, NCG], dt.float32)
        nc.sync.dma_start(preb[:], preb_d[:])
        convb = consts.tile([1, NL, F], dt.bfloat16)
        nc.sync.dma_start(convb[:], convb_d.unsqueeze(0))
        lng = consts.tile([128, NL, NCG], dt.float32)
        nc.sync.dma_start(lng[:], lng_d[:])
        lnb = consts.tile([128, NL, NCG], dt.float32)
        nc.sync.dma_start(lnb[:], lnb_d[:])
        projw = consts.tile([128, NCG, HALF * NP3], dt.bfloat16)
        nc.sync.dma_start(projw[:], projw_d[:])
        projb = consts.tile([128, HALF * NP3], dt.float32)
        nc.sync.dma_start(projb[:], projb_d[:])

        mask_bc = []
        mask_mg = []
        x_mega = []
        for e in range(BPC):
            if not trivial_mask:
                mb = consts.tile([128, T], dt.bfloat16, name=f"mask_bc{e}")
                nc.sync.dma_start(mb[:], maskbc[e])
                mask_bc.append(mb)
                mm = consts.tile([128, NTT], dt.float32, name=f"mask_mg{e}")
                nc.sync.dma_start(mm[:], maskmg[e])
                mask_mg.append(mm)
            xm = consts.tile([128, NTT, C_IN], dt.float32, name=f"x_mega{e}")
            nc.sync.dma_start(xm[:], x_mega_d[e])
            x_mega.append(xm)

        # spline params per example, [128, NTT, 58] f32
        params_pool = ctx.enter_context(tc.tile_pool(name="params", bufs=1))
        params = [params_pool.tile([128, NTT, HALF * NP3], dt.float32,
                                   name=f"params{e}") for e in range(BPC)]

        # ---------- psum + small work pools (live whole kernel) ----------
        psum_conv = ctx.enter_context(
            tc.tile_pool(name="psum_conv", bufs=2, space="PSUM"))
        psum_tp = ctx.enter_context(
            tc.tile_pool(name="psum_tp", bufs=3, space="PSUM"))
        psum_proj = ctx.enter_context(
            tc.tile_pool(name="psum_proj", bufs=2, space="PSUM"))
        stats = ctx.enter_context(tc.tile_pool(name="stats", bufs=2))
        xhat_pool = ctx.enter_context(
            tc.tile_pool(name="xhat", bufs=2 if trivial_mask else 1))

        def conv_block(psum, h_src, e, j, wt, bias_row):
            """psum[t,cout] = sum_{cg,k} h.T @ w (+ ones.T @ bias)"""
            nmm = NCG * KS
            i = 0
            for cg in range(NCG):
                for k in range(KS):
                    lhsT = h_src[e][cg][:, j * 128 + k: j * 128 + k + 128]
                    nt.matmul(psum[:], lhsT, wt[cg][k][:], start=(i == 0),
                              stop=(trivial_bias and i == nmm - 1))
                    i += 1
            if not trivial_bias:
                nt.matmul(psum[:], ones_col[:], bias_row,
                          start=False, stop=True)

        def ln_chunk(h_dst, e, j):
            """Scratch slot for tile j's raw conv output (token-major),
            parked inside the destination buffer: chunk m=j//4 of tile
            cg=j%4. gelu writes of tile j only overwrite chunk j//4, whose
            four parked tiles (quad 4*(j//4)..+3) are consumed first."""
            m = j // 4
            return h_dst[e][j % 4][:, 2 + 512 * m: 2 + 512 * (m + 1)]

        def ph1_units(e, l, src, dst, wt):
            """32 emission units: conv tile -> park chunk + sum/sumsq (DVE)."""
            sb = stats.tile([128, NTT], dt.float32, tag="sumb",
                            name=f"sumb{l}_{e}")
            sqb = stats.tile([128, NTT], dt.float32, tag="sqb",
                             name=f"sqb{l}_{e}")

            def unit(j):
                ps = psum_conv.tile([128, F], dt.float32, tag="ps",
                                    name=f"ps{l}_{e}_{j}")
                conv_block(ps, src, e, j, wt, convb[:, l, :])
                nv.tensor_scalar(ln_chunk(dst, e, j), ps[:], 0.0, None,
                                 Alu.add, Alu.add,
                                 accum_out=sb[:, j:j + 1])
                sqd = stats.tile([128, F], dt.bfloat16, tag="sqd",
                                 name=f"sqd{l}_{e}_{j}")
                ch = ln_chunk(dst, e, j)
                nv.scalar_tensor_tensor(sqd[:], ch, 1.0, ch,
                                        Alu.mult, Alu.mult,
                                        accum_out=sqb[:, j:j + 1])

            return [lambda j=j: unit(j) for j in range(NTT)], (sb, sqb)

        def ph23_units(e, l, dst, sb, sqb):
            """33 units: batched rstd prelude, then (quad,cg) tp+gelu."""
            state = {}

            def prelude():
                meanb = stats.tile([128, NTT], dt.float32, tag="meanb",
                                   name=f"meanb{l}_{e}")
                nv.tensor_scalar(meanb[:], sb[:], 1.0 / F, None, Alu.mult)
                varb = stats.tile([128, NTT], dt.float32, tag="varb",
                                  name=f"varb{l}_{e}")
                nv.scalar_tensor_tensor(varb[:], meanb[:], 0.0, meanb[:],
                                        Alu.add, Alu.mult)
                nv.scalar_tensor_tensor(varb[:], sqb[:], 1.0 / F, varb[:],
                                        Alu.mult, Alu.subtract)
                stdb = stats.tile([128, NTT], dt.float32, tag="stdb",
                                  name=f"stdb{l}_{e}")
                ns.activation(stdb[:], varb[:], Act.Sqrt, bias=eps_t[:])
                rstdb = stats.tile([128, NTT], dt.float32, tag="rstdb",
                                   name=f"rstdb{l}_{e}")
                nv.reciprocal(rstdb[:], stdb[:])
                state["m"] = meanb
                state["r"] = rstdb

            def unit(q, cg):
                if cg == 0:
                    state["xh"] = []
                    for jj in range(4):
                        j = 4 * q + jj
                        x = xhat_pool.tile([128, F], dt.bfloat16,
                                           tag=f"xh{jj}",
                                           name=f"xh{l}_{e}_{j}")
                        nv.tensor_scalar(x[:], ln_chunk(dst, e, j),
                                         state["m"][:, j:j + 1],
                                         state["r"][:, j:j + 1],
                                         Alu.subtract, Alu.mult)
                        state["xh"].append(x)
                xh = state["xh"]
                ptb = psum_tp.tile([128, 512], dt.bfloat16, tag="pt",
                                   name=f"pt{l}_{e}_{q}_{cg}")
                for jj in range(4):
                    nt.matmul(ptb[:, jj * 128:(jj + 1) * 128],
                              xh[jj][:, cg * 128:(cg + 1) * 128],
                              ident[:], is_transpose=True)
                ns.activation(
                    dst[e][cg][:, 2 + 512 * q: 2 + 512 * (q + 1)],
                    ptb[:], Act.Gelu,
                    bias=lnb[:, l, cg:cg + 1],
                    scale=lng[:, l, cg:cg + 1])

            return [prelude] + [lambda q=q, cg=cg: unit(q, cg)
                                for q in range(NTT // 4)
                                for cg in range(NCG)]

        def run_zip(a_units, b_units):
            """Interleave two unit lists (b may have a +1 prelude)."""
            if len(b_units) == len(a_units) + 1:
                b_units[0]()
                b_units = b_units[1:]
            n = max(len(a_units), len(b_units))
            for i in range(n):
                if i < len(a_units):
                    a_units[i]()
                if i < len(b_units):
                    b_units[i]()

        # ---------- activation ping-pong + weight pools ----------
        with tc.tile_pool(name="h_a", bufs=1) as h_a_pool:
            h_a = [[h_a_pool.tile([128, TPAD], dt.bfloat16, name=f"ha{e}_{cg}")
                    for cg in range(NCG)] for e in range(BPC)]
            with tc.tile_pool(name="h_b", bufs=1) as h_b_pool, \
                 tc.tile_pool(name="wpool", bufs=2) as wpool:
                h_b = [[h_b_pool.tile([128, TPAD], dt.bfloat16,
                                      name=f"hb{e}_{cg}")
                        for cg in range(NCG)] for e in range(BPC)]
                # zero the 2-col halos once; interiors are fully overwritten
                for hs in (h_a, h_b):
                    for e in range(BPC):
                        for cg in range(NCG):
                            nv.memset(hs[e][cg][:, 0:2], 0.0)
                            nv.memset(hs[e][cg][:, T + 2:T + 4], 0.0)

                def load_w(l):
                    wt = []
                    for cg in range(NCG):
                        row = []
                        for k in range(KS):
                            wb = 2 if trivial_mask else (
                                2 if (cg < 3 and k < 4) else 1)
                            w = wpool.tile([128, F], dt.bfloat16,
                                           tag=f"w{cg}_{k}", bufs=wb,
                                           name=f"w{l}_{cg}_{k}")
                            nc.sync.dma_start(
                                w[:],
                                convw_d[l, k, cg * 128:(cg + 1) * 128, :])
                            row.append(w)
                        wt.append(row)
                    return wt

                # ----- pre conv (1x1, K=2): x0 -> h_a, + bias, * mask -----
                wt_next = load_w(0)
                with tc.tile_pool(name="x0pool",
                                  bufs=2 if trivial_mask else 1) as x0pool:
                    for e in range(BPC):
                        for jb in range(NTB):
                            x0c = x0pool.tile([HALF, 512], dt.bfloat16,
                                              tag="x0c", name=f"x0c{e}_{jb}")
                            nc.sync.dma_start(
                                x0c[:], x0_bf[e][:, jb * 512:(jb + 1) * 512])
                            for cg in range(NCG):
                                ps = psum_conv.tile([128, 512], dt.float32,
                                                    tag="ps")
                                nt.matmul(ps[:],
                                          prew[:, cg * 128:(cg + 1) * 128],
                                          x0c[:])
                                hdst = h_a[e][cg][:, 2 + jb * 512:
                                                  2 + (jb + 1) * 512]
                                if trivial_bias and trivial_mask:
                                    ns.copy(hdst, ps[:])
                                elif trivial_mask:
                                    nv.tensor_scalar(hdst, ps[:],
                                                     preb[:, cg:cg + 1], None,
                                                     Alu.add)
                                else:
                                    nv.scalar_tensor_tensor(
                                        hdst, ps[:], preb[:, cg:cg + 1],
                                        mask_bc[e][:, jb * 512:(jb + 1) * 512],
                                        Alu.add, Alu.mult)

                # ----- 4 conv layers + proj, software-pipelined -----
                def proj_unit(e, j, h4):
                    pp = psum_proj.tile([128, HALF * NP3], dt.float32,
                                        tag="pp", name=f"pp{e}_{j}")
                    for cg in range(NCG):
                        nt.matmul(
                            pp[:],
                            h4[e][cg][:, 2 + j * 128: 2 + (j + 1) * 128],
                            projw[:, cg, :],
                            start=(cg == 0), stop=(cg == NCG - 1))
                    if trivial_bias and trivial_mask:
                        nv.tensor_copy(params[e][:, j, :], pp[:])
                    elif trivial_mask:
                        nv.tensor_tensor(params[e][:, j, :], pp[:],
                                         projb[:], Alu.add)
                    else:
                        tmp = stats.tile([128, HALF * NP3], dt.float32,
                                         tag="pj", name=f"pj{e}_{j}")
                        nv.tensor_tensor(tmp[:], pp[:], projb[:], Alu.add)
                        nv.tensor_scalar(params[e][:, j, :], tmp[:],
                                         mask_mg[e][:, j:j + 1], None,
                                         Alu.mult)

                wt_cur = wt_next
                prev_w = None
                src, dst = h_a, h_b
                for l in range(NL):
                    for e in range(BPC):
                        z, stt = ph1_units(e, l, src, dst, wt_cur)
                        if prev_w is None:
                            for u in z:
                                u()
                        else:
                            run_zip(z, prev_w)
                        prev_w = ph23_units(e, l, dst, *stt)
                        if e == 0 and l + 1 < NL:
                            wt_next = load_w(l + 1)
                        if (not trivial_mask) and e == BPC - 1 \
                                and l < NL - 1:
                            # general path: drain pipeline, apply mask
                            for u in prev_w:
                                u()
                            prev_w = None
                            for ee in range(BPC):
                                for cg in range(NCG):
                                    nv.tensor_tensor(
                                        dst[ee][cg][:, 2:2 + T],
                                        dst[ee][cg][:, 2:2 + T],
                                        mask_bc[ee][:], Alu.mult)
                    wt_cur = wt_next
                    src, dst = dst, src
                # final h (gelu4 out, unmasked) now in `src`
                h4 = src
                pu0 = [lambda j=j: proj_unit(0, j, h4) for j in range(NTT)]
                if prev_w is not None:
                    run_zip(pu0, prev_w)
                else:
                    for u in pu0:
                        u()
                for j in range(NTT):
                    proj_unit(1, j, h4)
            # h_b + wpool released; h_a no longer needed either

        # h_a released here
        # ---------- spline phase ----------
        sp = ctx.enter_context(tc.tile_pool(name="spline", bufs=2))

        for e in range(BPC):
            P4 = params[e][:].rearrange("p j (h k) -> p j h k", h=HALF)
            x1m = x_mega[e][:, :, HALF:C_IN]            # [128, 32, 2]

            def t3(tag, k=1):
                if k == 1:
                    return sp.tile([128, NTT, HALF], dt.float32, tag=tag,
                                   name=f"sp_{tag}_{e}")
                return sp.tile([128, NTT, HALF, k], dt.float32, tag=tag,
                               name=f"sp_{tag}_{e}")

            def bcast_in(a):
                """[128,32,2] -> broadcast over innermost K dim."""
                return a.unsqueeze(3)

            # --- softmax(w), softmax(h) (no max-sub; inputs are small) ---
            ew = t3("ew", NB)
            ns.activation(ew[:], P4[:, :, :, 0:NB], Act.Exp, scale=SSCALE)
            eh = t3("eh", NB)
            ns.activation(eh[:], P4[:, :, :, NB:2 * NB], Act.Exp, scale=SSCALE)
            sw = t3("sw")
            nv.tensor_reduce(sw[:], ew[:], Ax.X, Alu.add)
            sh = t3("sh")
            nv.tensor_reduce(sh[:], eh[:], Ax.X, Alu.add)
            rw = t3("rw")
            nv.reciprocal(rw[:], sw[:])
            rh = t3("rh")
            nv.reciprocal(rh[:], sh[:])

            # cumsum buffers [.., 18]: cols 8..17 hold data
            ca = t3("ca", 18)
            cb = t3("cb", 18)
            cc = t3("cc", 18)
            cd = t3("cd", 18)

            def norm_cumsum(ebuf, rbuf, a, b, minb):
                # a[..,8:18] = minb + (1-minb*NB) * e * r
                nv.scalar_tensor_tensor(a[:, :, :, 8:18], ebuf[:],
                                        0.0, rbuf.unsqueeze(3).broadcast_to((128, NTT, HALF, NB)),
                                        Alu.add, Alu.mult)
                nv.tensor_scalar(a[:, :, :, 8:18], a[:, :, :, 8:18],
                                 1.0 - minb * NB, minb, Alu.mult, Alu.add)
                nv.memset(a[:, :, :, 0:8], 0.0)
                nv.memset(b[:, :, :, 0:8], 0.0)
                nv.tensor_tensor(b[:, :, :, 8:18], a[:, :, :, 8:18],
                                 a[:, :, :, 7:17], Alu.add)
                nv.tensor_tensor(a[:, :, :, 8:18], b[:, :, :, 8:18],
                                 b[:, :, :, 6:16], Alu.add)
                nv.tensor_tensor(b[:, :, :, 8:18], a[:, :, :, 8:18],
                                 a[:, :, :, 4:14], Alu.add)
                nv.tensor_tensor(a[:, :, :, 8:18], b[:, :, :, 8:18],
                                 b[:, :, :, 0:10], Alu.add)
                # a[..,8:17] = cumsum_1..9 ; build knots
                return a

            cwsum = norm_cumsum(ew, rw, ca, cb, MIN_BW)
            chsum = norm_cumsum(eh, rh, cc, cd, MIN_BH)

            cw = t3("cw", NB + 1)
            nv.tensor_scalar(cw[:, :, :, 1:NB], cwsum[:, :, :, 8:17],
                             2.0 * TB, -TB, Alu.mult, Alu.add)
            nv.memset(cw[:, :, :, 0:1], -TB)
            nv.memset(cw[:, :, :, NB:NB + 1], TB)
            ch = t3("ch", NB + 1)
            nv.tensor_scalar(ch[:, :, :, 1:NB], chsum[:, :, :, 8:17],
                             2.0 * TB, -TB, Alu.mult, Alu.add)
            nv.memset(ch[:, :, :, 0:1], -TB)
            nv.memset(ch[:, :, :, NB:NB + 1], TB)

            wbin = t3("wbin", NB)
            nv.tensor_tensor(wbin[:], cw[:, :, :, 1:NB + 1],
                             cw[:, :, :, 0:NB], Alu.subtract)
            hbin = t3("hbin", NB)
            nv.tensor_tensor(hbin[:], ch[:, :, :, 1:NB + 1],
                             ch[:, :, :, 0:NB], Alu.subtract)

            # --- derivatives d[0..10]: ends exactly 1.0 ---
            dd = t3("dd", NB + 1)
            ns.activation(dd[:, :, :, 1:NB], P4[:, :, :, 2 * NB:NP3],
                          Act.Exp)
            nv.tensor_scalar(dd[:, :, :, 1:NB], dd[:, :, :, 1:NB],
                             1.0, None, Alu.add)
            ns.activation(dd[:, :, :, 1:NB], dd[:, :, :, 1:NB], Act.Ln)
            nv.tensor_scalar(dd[:, :, :, 1:NB], dd[:, :, :, 1:NB],
                             MIN_D, None, Alu.add)
            nv.memset(dd[:, :, :, 0:1], 1.0)
            nv.memset(dd[:, :, :, NB:NB + 1], 1.0)

            # --- bin index ---
            xin = t3("xin")
            nv.tensor_scalar(xin[:], x1m, -TB, TB, Alu.max, Alu.min)
            ge = t3("ge", NB)
            nv.tensor_tensor(ge[:], bcast_in(xin[:]).broadcast_to(
                (128, NTT, HALF, NB)), cw[:, :, :, 0:NB], Alu.is_ge)
            idx = t3("idx")
            nv.tensor_reduce(idx[:], ge[:], Ax.X, Alu.add)
            nv.tensor_scalar(idx[:], idx[:], -1.0, None, Alu.add)
            oh = t3("oh", NB)
            nv.tensor_tensor(
                oh[:],
                iota10[:].unsqueeze(1).unsqueeze(1).broadcast_to(
                    (128, NTT, HALF, NB)),
                bcast_in(idx[:]).broadcast_to((128, NTT, HALF, NB)),
                Alu.is_equal)

            # --- gathers via one-hot ---
            def gather(src_ap, tag):
                t = t3("gt_" + tag, NB)
                nv.tensor_tensor(t[:], src_ap, oh[:], Alu.mult)
                g = t3("g_" + tag)
                nv.tensor_reduce(g[:], t[:], Ax.X, Alu.add)
                return g

            g_cw = gather(cw[:, :, :, 0:NB], "cw")
            g_w = gather(wbin[:], "w")
            g_ch = gather(ch[:, :, :, 0:NB], "ch")
            g_h = gather(hbin[:], "h")
            g_d = gather(dd[:, :, :, 0:NB], "d")
            g_d1 = gather(dd[:, :, :, 1:NB + 1], "d1")

            # --- rational quadratic ---
            rgw = t3("rgw")
            nv.reciprocal(rgw[:], g_w[:])
            delta = t3("delta")
            nv.tensor_tensor(delta[:], g_h[:], rgw[:], Alu.mult)
            theta = t3("theta")
            nv.tensor_tensor(theta[:], xin[:], g_cw[:], Alu.subtract)
            nv.tensor_tensor(theta[:], theta[:], rgw[:], Alu.mult)
            omt = t3("omt")
            nv.tensor_scalar(omt[:], theta[:], -1.0, 1.0, Alu.mult, Alu.add)
            tomt = t3("tomt")
            nv.tensor_tensor(tomt[:], theta[:], omt[:], Alu.mult)
            th2 = t3("th2")
            nv.tensor_tensor(th2[:], theta[:], theta[:], Alu.mult)
            omt2 = t3("omt2")
            nv.tensor_tensor(omt2[:], omt[:], omt[:], Alu.mult)

            # num = g_h * (delta*th2 + g_d*tomt)
            t_a = t3("t_a")
            nv.tensor_tensor(t_a[:], delta[:], th2[:], Alu.mult)
            t_b = t3("t_b")
            nv.tensor_tensor(t_b[:], g_d[:], tomt[:], Alu.mult)
            nv.tensor_tensor(t_a[:], t_a[:], t_b[:], Alu.add)
            num = t3("num")
            nv.tensor_tensor(num[:], g_h[:], t_a[:], Alu.mult)
            # den = delta + (g_d + g_d1 - 2*delta) * tomt
            t_c = t3("t_c")
            nv.tensor_tensor(t_c[:], g_d[:], g_d1[:], Alu.add)
            nv.scalar_tensor_tensor(t_c[:], delta[:], -2.0, t_c[:],
                                    Alu.mult, Alu.add)
            nv.tensor_tensor(t_c[:], t_c[:], tomt[:], Alu.mult)
            den = t3("den")
            nv.tensor_tensor(den[:], t_c[:], delta[:], Alu.add)
            rden = t3("rden")
            nv.reciprocal(rden[:], den[:])
            outv = t3("outv")
            nv.tensor_tensor(outv[:], num[:], rden[:], Alu.mult)
            nv.tensor_tensor(outv[:], outv[:], g_ch[:], Alu.add)

            # dnum = delta^2*(g_d1*th2 + 2*delta*tomt + g_d*omt2)
            t_d = t3("t_d")
            nv.tensor_tensor(t_d[:], g_d1[:], th2[:], Alu.mult)
            t_e = t3("t_e")
            nv.scalar_tensor_tensor(t_e[:], delta[:], 2.0, tomt[:],
                                    Alu.mult, Alu.mult)
            nv.tensor_tensor(t_d[:], t_d[:], t_e[:], Alu.add)
            t_f = t3("t_f")
            nv.tensor_tensor(t_f[:], g_d[:], omt2[:], Alu.mult)
            nv.tensor_tensor(t_d[:], t_d[:], t_f[:], Alu.add)
            d2 = t3("d2")
            nv.tensor_tensor(d2[:], delta[:], delta[:], Alu.mult)
            nv.tensor_tensor(t_d[:], t_d[:], d2[:], Alu.mult)
            # lad = ln(dnum * rden^2)
            nv.tensor_tensor(t_d[:], t_d[:], rden[:], Alu.mult)
            nv.tensor_tensor(t_d[:], t_d[:], rden[:], Alu.mult)
            lad = t3("lad")
            ns.activation(lad[:], t_d[:], Act.Ln)

            # --- inside mask + select ---
            ins1 = t3("ins1")
            nv.tensor_scalar(ins1[:], x1m, -TB, None, Alu.is_ge)
            ins2 = t3("ins2")
            nv.tensor_scalar(ins2[:], x1m, TB, None, Alu.is_le)
            inside = t3("inside")
            nv.tensor_tensor(inside[:], ins1[:], ins2[:], Alu.mult)

            inside_i = sp.tile([128, NTT, HALF], dt.int32, tag="inside_i",
                               name=f"sp_inside_i_{e}")
            nv.tensor_copy(inside_i[:], inside[:])
            x1n = t3("x1n")
            nv.tensor_copy(x1n[:], x1m)
            nv.copy_predicated(x1n[:], inside_i[:], outv[:])

            # masked outputs
            if trivial_mask:
                x0m = x_mega[e][:, :, 0:HALF]
            else:
                mgb = mask_mg[e][:].unsqueeze(2).broadcast_to(
                    (128, NTT, HALF))
                x0m_t = t3("x0m")
                nv.tensor_tensor(x0m_t[:], x_mega[e][:, :, 0:HALF], mgb,
                                 Alu.mult)
                nv.tensor_tensor(x1n[:], x1n[:], mgb, Alu.mult)
                x0m = x0m_t[:]

            out_r = out_sh[e].rearrange("c (j p) -> p j c", p=128)
            for hc in range(HALF):
                nc.sync.dma_start(out_r[:, :, hc:hc + 1],
                                  x0m[:, :, hc:hc + 1])
                nc.sync.dma_start(out_r[:, :, HALF + hc:HALF + hc + 1],
                                  x1n[:, :, hc:hc + 1])

            # --- logdet ---
            ladm = t3("ladm")
            nv.tensor_tensor(ladm[:], lad[:], inside[:], Alu.mult)
            if not trivial_mask:
                nv.tensor_tensor(ladm[:], ladm[:], mgb, Alu.mult)
            ldp = sp.tile([128, 1], dt.float32, tag="ldp", name=f"ldp{e}")
            nv.tensor_reduce(ldp[:], ladm[:], Ax.XY, Alu.add)
            pl = psum_proj.tile([1, 1], dt.float32, tag="pl", bufs=1,
                                name=f"pl{e}")
            nt.matmul(pl[:], ldp[:], ones_f[:])
            lds = sp.tile([1, 1], dt.float32, tag="lds", name=f"lds{e}")
            ns.copy(lds[:], pl[:])
            nc.sync.dma_start(ld_sh[e], lds[:])

    nc.compile()
    return nc


_prog_cache = {}


def _triviality(inputs):
    """Host-visible input properties the program specializes on."""
    tb = (not np.any(np.asarray(inputs["pre_b"]))
          and not np.any(np.asarray(inputs["conv_b"]))
          and not np.any(np.asarray(inputs["proj_b"])))
    tm = bool(np.all(np.asarray(inputs["x_mask"]) == 1.0))
    return tb, tm


def _get_program(trivial_bias=True, trivial_mask=True):
    key = (trivial_bias, trivial_mask)
    if key not in _prog_cache:
        _prog_cache[key] = build_program(trivial_bias, trivial_mask)
    return _prog_cache[key]


def make_in_maps(x, x_mask, pre_w, pre_b, conv_w, conv_b, ln_g, ln_b,
                 proj_w, proj_b):
    x = np.asarray(x, np.float32)
    x_mask = np.asarray(x_mask, np.float32)
    prew = np.asarray(pre_w, np.float32).reshape(F, HALF).T.astype(BF16)
    preb = np.asarray(pre_b, np.float32).reshape(NCG, 128).T.copy()
    convw = np.transpose(np.asarray(conv_w, np.float32),
                         (0, 3, 2, 1)).astype(BF16).copy()
    convb = np.asarray(conv_b, np.float32).astype(BF16)
    lng = np.asarray(ln_g, np.float32).reshape(NL, NCG, 128).transpose(
        0, 2, 1).copy()
    lnb = np.asarray(ln_b, np.float32).reshape(NL, NCG, 128).transpose(
        0, 2, 1).copy()
    projw = np.asarray(proj_w, np.float32).reshape(HALF * NP3, F).T.astype(
        BF16).reshape(NCG, 128, HALF * NP3).transpose(1, 0, 2).copy()
    projb = np.tile(np.asarray(proj_b, np.float32)[None, :], (128, 1)).copy()
    ident = np.eye(128, dtype=BF16)
    iota = np.tile(np.arange(NB, dtype=np.float32)[None, :], (128, 1)).copy()
    onesc = np.ones((1, 128), BF16)
    onesf = np.ones((128, 1), np.float32)

    in_maps = []
    for c in range(NCORES):
        sl = slice(c * BPC, (c + 1) * BPC)
        xs = x[sl]
        ms = x_mask[sl]
        in_maps.append({
            "x_megah": np.ascontiguousarray(
                xs.reshape(BPC, C_IN, NTT, 128).transpose(0, 3, 2, 1)),
            "x0_bf": xs[:, :HALF, :].astype(BF16).copy(),
            "maskbc": np.tile(ms[:, 0:1, :].astype(BF16), (1, 128, 1)).copy(),
            "maskmg": np.ascontiguousarray(
                ms[:, 0, :].reshape(BPC, NTT, 128).transpose(0, 2, 1)),
            "prew": prew.copy(), "preb": preb, "convw": convw,
            "convb": convb,
            "lngh": np.ascontiguousarray(lng.transpose(1, 0, 2)),
            "lnbh": np.ascontiguousarray(lnb.transpose(1, 0, 2)),
            "projwh": projw, "projb": projb, "ident": ident,
            "iota10": iota, "onesc": onesc, "onesf": onesf,
        })
    return in_maps


def kernel(**inputs):
    from concourse.bass_utils import run_bass_kernel_spmd
    tb, tm = _triviality(inputs)
    nc = _get_program(tb, tm)
    in_maps = make_in_maps(**inputs)
    res = run_bass_kernel_spmd(nc, in_maps, list(range(NCORES))).results
    out = np.concatenate([r["out_sh"] for r in res], axis=0)
    logdet = np.concatenate([r["ld_sh"].reshape(BPC) for r in res], axis=0)
    return out.astype(np.float32), logdet.astype(np.float32)
